# revision 64
# baseline (speedup 1.0000x reference)
"""Trainium2 Bass kernel for nn_CAGpool (GNN message passing, CAG pooling).

Sharding: data-parallel over the 64 graph pairs -> 8 pairs (16 component
graphs of 512 nodes) per NeuronCore.  Message passing is dense matmul
against a per-graph 512x512 adjacency DMA'd from host as raw edge counts
(A+I); degree rows likewise host-derived from the integer edge lists.
Symmetric norm folded into C on-device with fused scalar_tensor_tensor.

Reductions over features use N=1 matmul columns; reductions over nodes use
stt-accum ops; the top-256 threshold comes from gpsimd kth_largest.
"""

import os
import numpy as np
import ml_dtypes

import concourse.bass as bass
import concourse.tile as tile
from concourse import bacc, mybir
from concourse.bass_utils import run_bass_kernel_spmd

F32 = mybir.dt.float32
BF16 = mybir.dt.bfloat16

NCORES = 8
B = 64
NPC = B // NCORES          # graph pairs per core (8)
NCG = 2 * NPC              # component graphs per core (16)
N = 512                    # nodes per component graph
K1 = 256
DEBUG = bool(int(os.environ.get("KERNEL_DEBUG", "0")))



def _layout(ent):
    offs, off = {}, 0
    for nm, w in ent:
        offs[nm] = (off, w)
        off += w
    return offs, off


WOFF, WF_TOT = _layout(
    [("W1", 128), ("W2", 128), ("W3", 128), ("Wgf", 128)]
    + [(f"Wg{i}", 384) for i in range(3)]
    + [(f"Wal{i}", 768) for i in range(6)]
    + [(f"Wf{i}", 128) for i in range(3)]
    + [("Wl1a", 128), ("Wl1b", 128), ("Wl2", 64), ("Wl3", 2),
       ("ones", 128), ("csel", 256), ("rsel", 2048)])
BOFF, BF_TOT = _layout(
    [("bfr", 128), ("balcol", 6), ("bl1col", 1), ("bl2col", 1),
     ("bl3col", 1), ("identf", 128), ("bcols", 3), ("deg", 512),
     ("degcol", 64)])


def _host_prep(inputs):
    """Per-core input maps. Index-structure prep only: dense adjacency counts
    and degree counts come straight from the integer edge lists."""
    x = np.asarray(inputs["x"], np.float32)

    s_loc, d_loc = {}, {}
    for comp, (sk, dk) in enumerate((("src_c1", "dst_c1"),
                                     ("src_c2", "dst_c2"))):
        base = (np.arange(B) * N)[:, None]
        s_loc[comp] = (np.asarray(inputs[sk]).reshape(B, -1) - base).astype(np.int64)
        d_loc[comp] = (np.asarray(inputs[dk]).reshape(B, -1) - base).astype(np.int64)

    in_maps = []
    loop = np.arange(N, dtype=np.int64)
    for c in range(NCORES):
        xT = np.empty((128, NCG * N), np.float32)
        deg = np.zeros((16, N), np.float32)
        idx_parts = []
        for comp in range(2):
            for gl in range(NPC):
                g = c * NPC + gl
                cg = comp * NPC + gl
                r0 = g * 2 * N + comp * N
                xT[:, cg * N:(cg + 1) * N] = x[r0:r0 + N].T
                s = np.concatenate([s_loc[comp][g], loop])
                d = np.concatenate([d_loc[comp][g], loop])
                idx = (s & 127) * (NCG * 2048) + cg * 2048 + (s >> 7) * 512 + d
                idx_parts.append(idx)
                deg[cg] = np.bincount(d_loc[comp][g], minlength=N) + 1.0
        cnt = np.bincount(np.concatenate(idx_parts),
                          minlength=128 * NCG * 2048)
        cdense = cnt.astype(ml_dtypes.bfloat16).reshape(128, NCG * 2048)

        wpack = np.zeros((128, WF_TOT), np.float32)

        def put(nm, arr):
            o, w = WOFF[nm]
            arr = np.asarray(arr, np.float32)
            wpack[: arr.shape[0], o:o + arr.shape[1]] = arr

        put("W1", inputs["W1"]); put("W2", inputs["W2"]); put("W3", inputs["W3"])
        put("Wgf", inputs["Wg_fin"])
        for i in range(3):
            put(f"Wg{i}", np.asarray(inputs["Wg_att"])[i * 128:(i + 1) * 128])
        for i in range(6):
            put(f"Wal{i}", np.asarray(inputs["Wal"])[i * 128:(i + 1) * 128])
        for i in range(3):
            put(f"Wf{i}", np.asarray(inputs["Wf"])[i * 128:(i + 1) * 128])
        put("Wl1a", np.asarray(inputs["Wl1"])[:128])
        put("Wl1b", np.asarray(inputs["Wl1"])[128:])
        put("Wl2", inputs["Wl2"])
        put("Wl3", inputs["Wl3"])
        put("ones", np.ones((128, 128), np.float32))
        csel = np.zeros((128, 256), np.float32)
        for cg in range(NCG):
            csel[:, cg * 16 + cg] = 1.0
        put("csel", csel)
        rsel = np.zeros((16, 2048), np.float32)
        for cg in range(16):
            rsel[cg, cg * 128:(cg + 1) * 128] = 1.0
        put("rsel", rsel)

        bpack = np.zeros((128, BF_TOT), np.float32)

        def putb(nm, arr):
            o, w = BOFF[nm]
            arr = np.asarray(arr, np.float32)
            bpack[: arr.shape[0], o:o + arr.shape[1]] = arr

        putb("bfr", np.broadcast_to(np.asarray(inputs["bf"])[None, :],
                                    (128, 128)))
        putb("balcol", np.asarray(inputs["bal"]).reshape(6, 128).T)
        putb("bl1col", np.asarray(inputs["bl1"])[:, None])
        putb("bl2col", np.asarray(inputs["bl2"])[:, None])
        putb("bl3col", np.asarray(inputs["bl3"])[:, None])
        putb("bcols", np.stack([np.asarray(inputs["b1"]),
                                np.asarray(inputs["b2"]),
                                np.asarray(inputs["b3"])], 1))
        putb("identf", np.eye(128, dtype=np.float32))
        putb("deg", deg)
        degcol = np.empty((128, 64), np.float32)
        for sblk in range(4):
            degcol[:, sblk * 16:(sblk + 1) * 16] = \
                deg[:, sblk * 128:(sblk + 1) * 128].T
        putb("degcol", degcol)

        in_maps.append({"xT": np.ascontiguousarray(xT.astype(ml_dtypes.bfloat16)),
                        "cdense": np.ascontiguousarray(cdense),
                        "wpack": wpack.astype(ml_dtypes.bfloat16),
                        "bpack": bpack})
    return in_maps


def _build():
    nc = bacc.Bacc("TRN2", target_bir_lowering=False, debug=False,
                   num_devices=NCORES)
    tin = {
        "xT": nc.dram_tensor("xT", [128, NCG * N], BF16, kind="ExternalInput"),
        "cdense": nc.dram_tensor("cdense", [128, NCG * 2048], BF16,
                                 kind="ExternalInput"),
        "wpack": nc.dram_tensor("wpack", [128, WF_TOT], BF16,
                                kind="ExternalInput"),
        "bpack": nc.dram_tensor("bpack", [128, BF_TOT], F32,
                                kind="ExternalInput"),
    }
    t_out = nc.dram_tensor("out", [2, NPC], F32, kind="ExternalOutput")
    dbg = {}
    if DEBUG:
        for nm, shape, dt in (
                ("C", [128, NCG * 2048], BF16), ("xcatT", [128, NCG * 1536], BF16),
                ("gpT", [128, 48], F32),
                ("scols", [128, 64], F32), ("thr", [1, 32], F32),
                ("mask", [16, N], F32), ("qrow", [16, N], F32),
                ("hp", [128, NCG * 512], BF16), ("pvTb", [128, 48], BF16)):
            dbg[nm] = nc.dram_tensor("dbg_" + nm, shape, dt,
                                     kind="ExternalOutput")
    with tile.TileContext(nc) as tc:
        _emit(nc, tc, tin, t_out, dbg)
    nc.compile()
    return nc


def _emit(nc, tc, tin, t_out, dbg):
    import contextlib
    ctx = contextlib.ExitStack()
    OP = mybir.AluOpType
    ACT = mybir.ActivationFunctionType

    const = ctx.enter_context(tc.tile_pool(name="const", bufs=1))
    rows = ctx.enter_context(tc.tile_pool(name="rows", bufs=1))
    work = ctx.enter_context(tc.tile_pool(name="work", bufs=4))
    scr = ctx.enter_context(tc.tile_pool(name="scr", bufs=3))
    psmm = ctx.enter_context(tc.tile_pool(name="psmm", bufs=4, space="PSUM"))
    pspt = ctx.enter_context(tc.tile_pool(name="pspt", bufs=2, space="PSUM"))
    pscol = ctx.enter_context(tc.tile_pool(name="pscol", bufs=1, space="PSUM"))
    psaux = ctx.enter_context(tc.tile_pool(name="psaux", bufs=1,
                                           space="PSUM"))

    wb = const.tile([128, WF_TOT], BF16, tag="wb")
    bp = const.tile([128, BF_TOT], F32, tag="bp")
    xTb = const.tile([128, NCG * N], BF16, tag="xTb")   # reused as hp later
    Call = const.tile([128, NCG * 2048], BF16, tag="Call")
    xcatT = const.tile([128, NCG * 1536], BF16, tag="xcatT")

    def W(nm):
        o, w = WOFF[nm]
        return wb[:, o:o + w]

    def Bc(nm):
        o, w = BOFF[nm]
        return bp[:, o:o + w]

    nc.sync.dma_start(bp[:], tin["bpack"].ap())
    # first two C chunks before the big weight/feature loads: the fold
    # pipeline (DVE) is the head-zone bottleneck and starts on chunk 0
    nc.sync.dma_start(Call[:, 0:4096], tin["cdense"].ap()[:, 0:4096])
    nc.gpsimd.dma_start(wb[:], tin["wpack"].ap())
    nc.scalar.dma_start(xTb[:], tin["xT"].ap())
    # remaining C chunks arrive per-2cg inside the fold/layer-1 loop below

    identf = Bc("identf")
    id16 = identf[0:16, 0:16]
    id1 = identf[0:1, 0:1]
    onescol = W("ones")[:, 0:1]

    def csel(cg):
        o, _ = WOFF["csel"]
        return wb[:, o + cg * 16: o + (cg + 1) * 16]

    def rself(cg):
        o, _ = WOFF["rsel"]
        return wb[0:16, o + cg * 128: o + (cg + 1) * 128]

    # ---- degree norm rows/cols --------------------------------------------
    deg_rows = Bc("deg")[0:16, :]
    sq_row = rows.tile([16, N], F32, tag="sq")
    nc.scalar.activation(sq_row[:], deg_rows, ACT.Sqrt)
    rsd_row = rows.tile([16, N], F32, tag="rsd")
    nc.vector.reciprocal(rsd_row[:], sq_row[:])
    rsd_rowb = rows.tile([16, N], BF16, tag="rsdb")
    nc.vector.tensor_copy(rsd_rowb[:], rsd_row[:])

    rsdcol = const.tile([128, 64], F32, tag="rsdcol")
    sqcol = const.tile([128, 64], F32, tag="sqcol")
    nc.scalar.activation(sqcol[:], Bc("degcol"), ACT.Sqrt)
    nc.vector.reciprocal(rsdcol[:], sqcol[:])

    # ---- fold norm into C + layer 1, pipelined behind chunked C DMA -------
    meanT = const.tile([128, 48], F32, tag="meanT")
    meanT2 = const.tile([128, 48], F32, tag="meanT2")
    zeros256 = const.tile([128, 256], BF16, tag="zeros256")
    nc.vector.memset(zeros256[:], 0.0)

    def layer(l, cg):
        wl = W(("W1", "W2", "W3")[l])
        bcol = Bc("bcols")[:, l:l + 1]
        pxw = psmm.tile([128, 512], F32, tag="mm")
        for nt in range(4):
            if l == 0:
                lhsT = xTb[:, cg * N + nt * 128: cg * N + (nt + 1) * 128]
            else:
                lhsT = xcatT[:, cg * 1536 + (l - 1) * 512 + nt * 128:
                             cg * 1536 + (l - 1) * 512 + (nt + 1) * 128]
            nc.tensor.matmul(pxw[:, nt * 128:(nt + 1) * 128], lhsT=lhsT,
                             rhs=wl, start=True, stop=True)
        xws = scr.tile([128, 512], BF16, tag="xws")
        if l == 0:
            nc.scalar.activation(xws[:], pxw[:], ACT.Copy)
        else:
            nc.vector.tensor_copy(xws[:, 0:256], pxw[:, 0:256])
            nc.scalar.activation(xws[:, 256:512], pxw[:, 256:512], ACT.Copy)
        ph = psmm.tile([128, 512], F32, tag="mm")
        for sblk in range(4):
            nc.tensor.matmul(
                ph[:],
                lhsT=xws[:, sblk * 128:(sblk + 1) * 128],
                rhs=Call[:, cg * 2048 + sblk * 512:
                         cg * 2048 + (sblk + 1) * 512],
                start=(sblk == 0), stop=(sblk == 3))
        out_sl = xcatT[:, cg * 1536 + l * 512: cg * 1536 + (l + 1) * 512]
        if l == 0:
            nc.scalar.activation(
                out_sl, ph[:], ACT.Relu, bias=bcol,
                accum_out=meanT[:, l * 16 + cg: l * 16 + cg + 1])
            nc.vector.memset(meanT2[:, l * 16 + cg: l * 16 + cg + 1], 0.0)
        else:
            nc.scalar.activation(
                out_sl[:, 0:256], ph[:, 0:256], ACT.Relu, bias=bcol,
                accum_out=meanT[:, l * 16 + cg: l * 16 + cg + 1])
            nc.vector.scalar_tensor_tensor(
                out_sl[:, 256:512], ph[:, 256:512], bcol, zeros256[:],
                op0=OP.add, op1=OP.max,
                accum_out=meanT2[:, l * 16 + cg: l * 16 + cg + 1])

    def fold_l0(cg):
        if cg % 2 == 0 and cg >= 2:
            nc.sync.dma_start(Call[:, cg * 2048:(cg + 2) * 2048],
                              tin["cdense"].ap()[:, cg * 2048:(cg + 2) * 2048])
        pbps = psmm.tile([128, N], F32, tag="mm")
        nc.tensor.matmul(pbps[:], lhsT=rself(cg), rhs=rsd_rowb[:],
                         start=True, stop=True)
        for sblk in range(4):
            sl = Call[:, cg * 2048 + sblk * 512: cg * 2048 + (sblk + 1) * 512]
            nc.vector.scalar_tensor_tensor(
                sl, sl, rsdcol[:, sblk * 16 + cg: sblk * 16 + cg + 1],
                pbps[:], op0=OP.mult, op1=OP.mult)
        layer(0, cg)

    # layers 2+3 interleaved (software-pipelined); per-cg after layer 3:
    # c = tanh(mean @ Wg), alpha cols, sigmoid, node-major xcat copy via
    # DMA transpose, then gp via N=1 matmuls
    meanTb = rows.tile([128, 48], BF16, tag="meanTb")
    msum = rows.tile([128, 48], F32, tag="msum")
    cTb = rows.tile([128, 48], BF16, tag="cTb")
    asigb = rows.tile([128, 64], BF16, tag="asigb")
    pca = pscol.tile([128, 64], F32, tag="cols")
    gpTp = psaux.tile([128, 64], F32, tag="aux")

    def attn(cg):
        nc.vector.tensor_tensor(msum[:, cg::16], meanT[:, cg::16],
                                meanT2[:, cg::16], op=OP.add)
        nc.vector.tensor_scalar(meanTb[:, cg::16], msum[:, cg::16],
                                1.0 / N, None, op0=OP.mult)
        pc = pspt.tile([128, 128], F32, tag="pt")
        for fo in range(3):
            for fi in range(3):
                nc.tensor.matmul(
                    pc[:, fo:fo + 1],
                    lhsT=W(f"Wg{fi}")[:, fo * 128:(fo + 1) * 128],
                    rhs=meanTb[:, fi * 16 + cg: fi * 16 + cg + 1],
                    start=(fi == 0), stop=(fi == 2))
        nc.scalar.activation(cTb[:, cg::16], pc[:, 0:3], ACT.Tanh)
        for nt in range(4):
            for ch in range(3):
                nc.tensor.matmul(
                    pca[:, nt * 16 + cg: nt * 16 + cg + 1],
                    lhsT=xcatT[:, cg * 1536 + ch * 512 + nt * 128:
                               cg * 1536 + ch * 512 + (nt + 1) * 128],
                    rhs=cTb[:, ch * 16 + cg: ch * 16 + cg + 1],
                    start=(ch == 0), stop=(ch == 2))
        nc.scalar.activation(asigb[:, cg::16], pca[:, cg::16], ACT.Sigmoid)
        xcN = scr.tile([128, 1536], BF16, tag="xcN")
        nc.sync.dma_start_transpose(
            xcN[:].rearrange("p (c f) -> p c f", c=12, f=128),
            xcatT[:, cg * 1536:(cg + 1) * 1536])
        for ch in range(3):
            for nt in range(4):
                nc.tensor.matmul(
                    gpTp[:, ch * 16 + cg: ch * 16 + cg + 1],
                    lhsT=xcN[:, (ch * 4 + nt) * 128:(ch * 4 + nt + 1) * 128],
                    rhs=asigb[:, nt * 16 + cg: nt * 16 + cg + 1],
                    start=(nt == 0), stop=(nt == 3))

    for i in range(NCG + 4):
        if i < NCG:
            fold_l0(i)
        if 2 <= i < NCG + 2:
            layer(1, i - 2)
        if i >= 4:
            layer(2, i - 4)
            attn(i - 4)
    if DEBUG:
        nc.sync.dma_start(dbg["C"].ap(), Call[:])
    if DEBUG:
        nc.sync.dma_start(dbg["xcatT"].ap(), xcatT[:])
        nc.sync.dma_start(dbg["gpT"].ap(), gpTp[:, 0:48])
    gpT = gpTp

    # ---- pv = att_lin(concat(gp1, gp2)) -----------------------------------
    gpcatTb = rows.tile([128, 48], BF16, tag="gpcatTb")
    for j in range(6):
        comp, ch = j // 3, j % 3
        nc.vector.tensor_copy(
            gpcatTb[:, j * 8:(j + 1) * 8],
            gpT[:, ch * 16 + comp * 8: ch * 16 + comp * 8 + 8])
    pvTb = rows.tile([128, 48], BF16, tag="pvTb")
    for co in range(6):
        pp = pspt.tile([128, 128], F32, tag="pt")
        for ci in range(6):
            nc.tensor.matmul(pp[:, 0:8],
                             lhsT=W(f"Wal{ci}")[:, co * 128:(co + 1) * 128],
                             rhs=gpcatTb[:, ci * 8:(ci + 1) * 8],
                             start=(ci == 0), stop=(ci == 5))
        nc.vector.tensor_scalar(pvTb[:, co * 8:(co + 1) * 8], pp[:, 0:8],
                                Bc("balcol")[:, co:co + 1], None, op0=OP.add)
    if DEBUG:
        nc.sync.dma_start(dbg["pvTb"].ap(), pvTb[:])

    # ---- 1/||pv|| per graph ------------------------------------------------
    pnn = pspt.tile([128, 128], F32, tag="pt")
    for j in range(16):
        comp, gl = j // 8, j % 8
        for ci in range(3):
            col = pvTb[:, (comp * 3 + ci) * 8 + gl: (comp * 3 + ci) * 8 + gl + 1]
            nc.tensor.matmul(pnn[0:1, j:j + 1], lhsT=col, rhs=col,
                             start=(ci == 0), stop=(ci == 2))
    nnrow = rows.tile([1, 16], F32, tag="nnrow")
    nc.vector.tensor_copy(nnrow[:], pnn[0:1, 0:16])
    sqnrow = rows.tile([1, 16], F32, tag="sqnrow")
    nc.scalar.activation(sqnrow[:], nnrow[:], ACT.Sqrt)
    rsnrow = rows.tile([1, 16], F32, tag="rsnrow")
    nc.vector.reciprocal(rsnrow[:], sqnrow[:])
    ptn = pspt.tile([128, 128], F32, tag="pt")
    nc.tensor.transpose(ptn[0:16, 0:1], rsnrow[:], id1)
    rsncol = rows.tile([16, 1], F32, tag="rsncol")
    nc.vector.tensor_copy(rsncol[:], ptn[0:16, 0:1])

    # ---- scores (cg-major cols for kth_largest) ---------------------------
    pcs = pscol.tile([128, 64], F32, tag="cols")
    for cg in range(NCG):
        comp, gl = cg // NPC, cg % NPC
        for nt in range(4):
            for ci in range(3):
                nc.tensor.matmul(
                    pcs[:, cg * 4 + nt: cg * 4 + nt + 1],
                    lhsT=xcatT[:, cg * 1536 + ci * 512 + nt * 128:
                               cg * 1536 + ci * 512 + (nt + 1) * 128],
                    rhs=pvTb[:, (comp * 3 + ci) * 8 + gl:
                             (comp * 3 + ci) * 8 + gl + 1],
                    start=(ci == 0), stop=(ci == 2))
    scols = rows.tile([128, 64], F32, tag="scols")
    nc.vector.tensor_copy(scols[:], pcs[:])
    if DEBUG:
        nc.sync.dma_start(dbg["scols"].ap(), scols[:])

    thr = rows.tile([1, 32], F32, tag="thr")
    for g in range(16):
        nc.gpsimd.kth_largest(thr[0:1, 2 * g:2 * g + 2],
                              scols[:, g * 4:(g + 1) * 4],
                              n_per_lane=4, k=256, quantile=0.5005)
    if DEBUG:
        nc.sync.dma_start(dbg["thr"].ap(), thr[:])
    ptt = pspt.tile([128, 128], F32, tag="pt")
    nc.tensor.transpose(ptt[0:16, 0:1], thr[0:1, 0::2], id1)
    thrcol = rows.tile([16, 1], F32, tag="thrcol")
    nc.vector.tensor_copy(thrcol[:], ptt[0:16, 0:1])

    score_row = rows.tile([16, N], F32, tag="score")
    sig_row = rows.tile([16, N], F32, tag="sig")
    for nt in range(4):
        pt = pspt.tile([128, 128], F32, tag="pt")
        nc.tensor.transpose(pt[0:16, :], scols[:, nt::4], identf)
        nc.vector.tensor_copy(score_row[:, nt * 128:(nt + 1) * 128],
                              pt[0:16, :])
        nc.scalar.activation(sig_row[:, nt * 128:(nt + 1) * 128],
                             pt[0:16, :], ACT.Sigmoid, scale=rsncol[:])

    mask_row = rows.tile([16, N], F32, tag="mask")
    nc.vector.tensor_scalar(mask_row[:], score_row[:], thrcol[:], None,
                            op0=OP.is_gt)
    if DEBUG:
        nc.sync.dma_start(dbg["mask"].ap(), mask_row[:])

    # ---- pooled degree -----------------------------------------------------
    mcol = const.tile([128, 64], F32, tag="mcol")
    for sblk in range(4):
        pt = pspt.tile([128, 128], F32, tag="pt")
        nc.tensor.transpose(pt[:, 0:16],
                            mask_row[:, sblk * 128:(sblk + 1) * 128], id16)
        nc.vector.tensor_copy(mcol[:, sblk * 16:(sblk + 1) * 16], pt[:, 0:16])
    msqcol = const.tile([128, 64], F32, tag="msqcol")
    nc.vector.tensor_tensor(msqcol[:], mcol[:], sqcol[:], op=OP.mult)

    ps_d2 = psaux.tile([16, N], F32, tag="aux")
    for cg in range(NCG):
        for sblk in range(4):
            mlh = work.tile([128, 16], BF16, tag="mlh")
            nc.vector.tensor_scalar(
                mlh[:], csel(cg),
                msqcol[:, sblk * 16 + cg: sblk * 16 + cg + 1], None,
                op0=OP.mult)
            nc.tensor.matmul(
                ps_d2[:], lhsT=mlh[:],
                rhs=Call[:, cg * 2048 + sblk * 512: cg * 2048 + (sblk + 1) * 512],
                start=(cg == 0 and sblk == 0),
                stop=(cg == NCG - 1 and sblk == 3))
    sqm_row = rows.tile([16, N], F32, tag="sqm")
    nc.vector.tensor_tensor(sqm_row[:], sq_row[:], mask_row[:], op=OP.mult)
    d2a = rows.tile([16, N], F32, tag="d2a")
    nc.vector.tensor_tensor(d2a[:], ps_d2[:], sqm_row[:], op=OP.mult)
    d2b = rows.tile([16, N], F32, tag="d2b")
    nc.vector.tensor_tensor(d2b[:], d2a[:], mask_row[:], op=OP.subtract)
    sq2_row = rows.tile([16, N], F32, tag="sq2")
    nc.scalar.activation(sq2_row[:], d2b[:], ACT.Sqrt, bias=1.0)
    rsd2_row = rows.tile([16, N], F32, tag="rsd2")
    nc.vector.reciprocal(rsd2_row[:], sq2_row[:])
    mr2_row = rows.tile([16, N], F32, tag="mr2")
    nc.vector.tensor_tensor(mr2_row[:], rsd2_row[:], mask_row[:], op=OP.mult)
    q_row = rows.tile([16, N], F32, tag="qrow")
    nc.vector.tensor_tensor(q_row[:], mr2_row[:], sq_row[:], op=OP.mult)
    gate2_row = rows.tile([16, N], F32, tag="gate2")
    nc.vector.tensor_tensor(gate2_row[:], sig_row[:], q_row[:], op=OP.mult)
    if DEBUG:
        nc.sync.dma_start(dbg["qrow"].ap(), q_row[:])

    qcol = const.tile([128, 64], F32, tag="qcol")
    g2col = const.tile([128, 64], F32, tag="g2col")
    for sblk in range(4):
        pt = pspt.tile([128, 128], F32, tag="pt")
        nc.tensor.transpose(pt[:, 0:16],
                            q_row[:, sblk * 128:(sblk + 1) * 128], id16)
        nc.vector.tensor_copy(qcol[:, sblk * 16:(sblk + 1) * 16], pt[:, 0:16])
        pt2 = pspt.tile([128, 128], F32, tag="pt")
        nc.tensor.transpose(pt2[:, 0:16],
                            gate2_row[:, sblk * 128:(sblk + 1) * 128], id16)
        nc.vector.tensor_copy(g2col[:, sblk * 16:(sblk + 1) * 16],
                              pt2[:, 0:16])

    # ---- pooled conv (node-major) + fused final attention pool ------------
    hpall = xTb  # xTb fully consumed by layer 1
    bfr = Bc("bfr")
    ps_mg = pscol.tile([128, 64], F32, tag="cols")  # cols 0:16 mean, 16:32 g
    mT2b = rows.tile([128, 16], BF16, tag="mT2b")
    c2b = rows.tile([128, 16], BF16, tag="c2b")
    a4 = rows.tile([128, 64], BF16, tag="a4")
    for cg in range(NCG):
        pxp = psmm.tile([128, 512], F32, tag="mm")
        for nt in range(4):
            for ci in range(3):
                nc.tensor.matmul(
                    pxp[:, nt * 128:(nt + 1) * 128],
                    lhsT=xcatT[:, cg * 1536 + ci * 512 + nt * 128:
                               cg * 1536 + ci * 512 + (nt + 1) * 128],
                    rhs=W(f"Wf{ci}"), start=(ci == 0), stop=(ci == 2))
        xwps = scr.tile([128, 512], BF16, tag="xwps")
        for nt in range(4):
            nc.scalar.activation(
                xwps[:, nt * 128:(nt + 1) * 128],
                pxp[:, nt * 128:(nt + 1) * 128], ACT.Copy,
                scale=g2col[:, nt * 16 + cg: nt * 16 + cg + 1])
        pm = psmm.tile([128, 512], F32, tag="mm")
        for dt in range(4):
            for sblk in range(4):
                nc.tensor.matmul(
                    pm[:, dt * 128:(dt + 1) * 128],
                    lhsT=Call[:, cg * 2048 + sblk * 512 + dt * 128:
                              cg * 2048 + sblk * 512 + (dt + 1) * 128],
                    rhs=xwps[:, sblk * 128:(sblk + 1) * 128],
                    start=(sblk == 0), stop=(sblk == 3))
        hp = hpall[:, cg * 512:(cg + 1) * 512]
        y2 = scr.tile([128, 512], BF16, tag="y2")
        for dt in range(4):
            nc.vector.scalar_tensor_tensor(
                y2[:, dt * 128:(dt + 1) * 128],
                pm[:, dt * 128:(dt + 1) * 128],
                qcol[:, dt * 16 + cg: dt * 16 + cg + 1], bfr,
                op0=OP.mult, op1=OP.add)
            nc.vector.tensor_scalar(
                hp[:, dt * 128:(dt + 1) * 128],
                y2[:, dt * 128:(dt + 1) * 128], 0.0,
                mcol[:, dt * 16 + cg: dt * 16 + cg + 1],
                op0=OP.max, op1=OP.mult)
        for dt in range(4):
            nc.tensor.matmul(ps_mg[:, cg:cg + 1],
                             lhsT=hp[:, dt * 128:(dt + 1) * 128],
                             rhs=onescol, start=(dt == 0), stop=(dt == 3))
        # per-graph c2 = tanh(mean @ Wg_fin); alpha2 via transposed hp
        pt2 = pspt.tile([128, 128], F32, tag="pt")
        nc.vector.tensor_scalar(mT2b[:, cg:cg + 1], ps_mg[:, cg:cg + 1],
                                1.0 / K1, None, op0=OP.mult)
        nc.tensor.matmul(pt2[:, 4:5], lhsT=W("Wgf"), rhs=mT2b[:, cg:cg + 1],
                         start=True, stop=True)
        nc.scalar.activation(c2b[:, cg:cg + 1], pt2[:, 4:5], ACT.Tanh)
        hpT = scr.tile([128, 512], BF16, tag="hpT")
        nc.sync.dma_start_transpose(
            hpT[:].rearrange("p (c f) -> p c f", c=4, f=128), hp[:])
        for dt in range(4):
            nc.tensor.matmul(pt2[:, dt:dt + 1],
                             lhsT=hpT[:, dt * 128:(dt + 1) * 128],
                             rhs=c2b[:, cg:cg + 1], start=True, stop=True)
        nc.scalar.activation(a4[:, cg * 4:(cg + 1) * 4], pt2[:, 0:4],
                             ACT.Sigmoid)
        for dt in range(4):
            nc.tensor.matmul(ps_mg[:, 16 + cg: 16 + cg + 1],
                             lhsT=hp[:, dt * 128:(dt + 1) * 128],
                             rhs=a4[:, cg * 4 + dt: cg * 4 + dt + 1],
                             start=(dt == 0), stop=(dt == 3))
    if DEBUG:
        nc.sync.dma_start(dbg["hp"].ap(), hpall[:])

    # ---- head MLP ----------------------------------------------------------
    pcat = rows.tile([128, 16], BF16, tag="pcat")
    nc.vector.tensor_copy(pcat[:], ps_mg[:, 16:32])
    p1 = pspt.tile([128, 128], F32, tag="pt")
    nc.tensor.matmul(p1[:, 0:NPC], lhsT=W("Wl1a"), rhs=pcat[:, 0:NPC],
                     start=True, stop=False)
    nc.tensor.matmul(p1[:, 0:NPC], lhsT=W("Wl1b"), rhs=pcat[:, NPC:2 * NPC],
                     start=False, stop=True)
    o1 = rows.tile([128, NPC], BF16, tag="o1")
    nc.scalar.activation(o1[:], p1[:, 0:NPC], ACT.Relu, bias=Bc("bl1col")[:])
    p2 = pspt.tile([128, 128], F32, tag="pt")
    nc.tensor.matmul(p2[0:64, 0:NPC], lhsT=W("Wl2"), rhs=o1[:], start=True,
                     stop=True)
    o2 = rows.tile([64, NPC], BF16, tag="o2")
    nc.scalar.activation(o2[:], p2[0:64, 0:NPC], ACT.Relu,
                         bias=Bc("bl2col")[0:64, :])
    p3 = pspt.tile([128, 128], F32, tag="pt")
    nc.tensor.matmul(p3[0:2, 0:NPC], lhsT=W("Wl3")[0:64, :], rhs=o2[:],
                     start=True, stop=True)
    o3 = rows.tile([2, NPC], F32, tag="o3")
    nc.vector.tensor_scalar(o3[:], p3[0:2, 0:NPC], Bc("bl3col")[0:2, :],
                            None, op0=OP.add)
    nc.sync.dma_start(t_out.ap(), o3[:])
    ctx.close()


_NC_CACHE = {}


def _get_nc():
    key = (DEBUG,)
    if key not in _NC_CACHE:
        _NC_CACHE[key] = _build()
    return _NC_CACHE[key]


def kernel(**inputs):
    in_maps = _host_prep(inputs)
    nc = _get_nc()
    res = run_bass_kernel_spmd(nc, in_maps, core_ids=list(range(NCORES)))
    out = np.empty((B, 2), np.float32)
    for c in range(NCORES):
        out[c * NPC:(c + 1) * NPC] = res.results[c]["out"].T
    kernel._last = res
    kernel._nc = nc
    return out


# revision 65
# speedup vs baseline: 1.0002x; 1.0002x over previous
"""Trainium2 Bass kernel for nn_CAGpool (GNN message passing, CAG pooling).

Sharding: data-parallel over the 64 graph pairs -> 8 pairs (16 component
graphs of 512 nodes) per NeuronCore.  Message passing is dense matmul
against a per-graph 512x512 adjacency DMA'd from host as raw edge counts
(A+I); degree rows likewise host-derived from the integer edge lists.
Symmetric norm folded into C on-device with fused scalar_tensor_tensor.

Reductions over features use N=1 matmul columns; reductions over nodes use
stt-accum ops; the top-256 threshold comes from gpsimd kth_largest.
"""

import os
import numpy as np
import ml_dtypes

import concourse.bass as bass
import concourse.tile as tile
from concourse import bacc, mybir
from concourse.bass_utils import run_bass_kernel_spmd

F32 = mybir.dt.float32
BF16 = mybir.dt.bfloat16

NCORES = 8
B = 64
NPC = B // NCORES          # graph pairs per core (8)
NCG = 2 * NPC              # component graphs per core (16)
N = 512                    # nodes per component graph
K1 = 256
DEBUG = bool(int(os.environ.get("KERNEL_DEBUG", "0")))



def _layout(ent):
    offs, off = {}, 0
    for nm, w in ent:
        offs[nm] = (off, w)
        off += w
    return offs, off


WOFF, WF_TOT = _layout(
    [("W1", 128), ("W2", 128), ("W3", 128), ("Wgf", 128)]
    + [(f"Wg{i}", 384) for i in range(3)]
    + [(f"Wal{i}", 768) for i in range(6)]
    + [(f"Wf{i}", 128) for i in range(3)]
    + [("Wl1a", 128), ("Wl1b", 128), ("Wl2", 64), ("Wl3", 2),
       ("ones", 128), ("csel", 256), ("rsel", 2048)])
BOFF, BF_TOT = _layout(
    [("bfr", 128), ("balcol", 6), ("bl1col", 1), ("bl2col", 1),
     ("bl3col", 1), ("identf", 128), ("bcols", 3), ("deg", 512),
     ("degcol", 64)])


def _host_prep(inputs):
    """Per-core input maps. Index-structure prep only: dense adjacency counts
    and degree counts come straight from the integer edge lists."""
    x = np.asarray(inputs["x"], np.float32)

    s_loc, d_loc = {}, {}
    for comp, (sk, dk) in enumerate((("src_c1", "dst_c1"),
                                     ("src_c2", "dst_c2"))):
        base = (np.arange(B) * N)[:, None]
        s_loc[comp] = (np.asarray(inputs[sk]).reshape(B, -1) - base).astype(np.int64)
        d_loc[comp] = (np.asarray(inputs[dk]).reshape(B, -1) - base).astype(np.int64)

    in_maps = []
    loop = np.arange(N, dtype=np.int64)
    for c in range(NCORES):
        xT = np.empty((128, NCG * N), np.float32)
        deg = np.zeros((16, N), np.float32)
        idx_parts = []
        for comp in range(2):
            for gl in range(NPC):
                g = c * NPC + gl
                cg = comp * NPC + gl
                r0 = g * 2 * N + comp * N
                xT[:, cg * N:(cg + 1) * N] = x[r0:r0 + N].T
                s = np.concatenate([s_loc[comp][g], loop])
                d = np.concatenate([d_loc[comp][g], loop])
                idx = (s & 127) * (NCG * 2048) + cg * 2048 + (s >> 7) * 512 + d
                idx_parts.append(idx)
                deg[cg] = np.bincount(d_loc[comp][g], minlength=N) + 1.0
        cnt = np.bincount(np.concatenate(idx_parts),
                          minlength=128 * NCG * 2048)
        cdense = cnt.astype(ml_dtypes.bfloat16).reshape(128, NCG * 2048)

        wpack = np.zeros((128, WF_TOT), np.float32)

        def put(nm, arr):
            o, w = WOFF[nm]
            arr = np.asarray(arr, np.float32)
            wpack[: arr.shape[0], o:o + arr.shape[1]] = arr

        put("W1", inputs["W1"]); put("W2", inputs["W2"]); put("W3", inputs["W3"])
        put("Wgf", inputs["Wg_fin"])
        for i in range(3):
            put(f"Wg{i}", np.asarray(inputs["Wg_att"])[i * 128:(i + 1) * 128])
        for i in range(6):
            put(f"Wal{i}", np.asarray(inputs["Wal"])[i * 128:(i + 1) * 128])
        for i in range(3):
            put(f"Wf{i}", np.asarray(inputs["Wf"])[i * 128:(i + 1) * 128])
        put("Wl1a", np.asarray(inputs["Wl1"])[:128])
        put("Wl1b", np.asarray(inputs["Wl1"])[128:])
        put("Wl2", inputs["Wl2"])
        put("Wl3", inputs["Wl3"])
        put("ones", np.ones((128, 128), np.float32))
        csel = np.zeros((128, 256), np.float32)
        for cg in range(NCG):
            csel[:, cg * 16 + cg] = 1.0
        put("csel", csel)
        rsel = np.zeros((16, 2048), np.float32)
        for cg in range(16):
            rsel[cg, cg * 128:(cg + 1) * 128] = 1.0
        put("rsel", rsel)

        bpack = np.zeros((128, BF_TOT), np.float32)

        def putb(nm, arr):
            o, w = BOFF[nm]
            arr = np.asarray(arr, np.float32)
            bpack[: arr.shape[0], o:o + arr.shape[1]] = arr

        putb("bfr", np.broadcast_to(np.asarray(inputs["bf"])[None, :],
                                    (128, 128)))
        putb("balcol", np.asarray(inputs["bal"]).reshape(6, 128).T)
        putb("bl1col", np.asarray(inputs["bl1"])[:, None])
        putb("bl2col", np.asarray(inputs["bl2"])[:, None])
        putb("bl3col", np.asarray(inputs["bl3"])[:, None])
        putb("bcols", np.stack([np.asarray(inputs["b1"]),
                                np.asarray(inputs["b2"]),
                                np.asarray(inputs["b3"])], 1))
        putb("identf", np.eye(128, dtype=np.float32))
        putb("deg", deg)
        degcol = np.empty((128, 64), np.float32)
        for sblk in range(4):
            degcol[:, sblk * 16:(sblk + 1) * 16] = \
                deg[:, sblk * 128:(sblk + 1) * 128].T
        putb("degcol", degcol)

        in_maps.append({"xT": np.ascontiguousarray(xT.astype(ml_dtypes.bfloat16)),
                        "cdense": np.ascontiguousarray(cdense),
                        "wpack": wpack.astype(ml_dtypes.bfloat16),
                        "bpack": bpack})
    return in_maps


def _build():
    nc = bacc.Bacc("TRN2", target_bir_lowering=False, debug=False,
                   num_devices=NCORES)
    tin = {
        "xT": nc.dram_tensor("xT", [128, NCG * N], BF16, kind="ExternalInput"),
        "cdense": nc.dram_tensor("cdense", [128, NCG * 2048], BF16,
                                 kind="ExternalInput"),
        "wpack": nc.dram_tensor("wpack", [128, WF_TOT], BF16,
                                kind="ExternalInput"),
        "bpack": nc.dram_tensor("bpack", [128, BF_TOT], F32,
                                kind="ExternalInput"),
    }
    t_out = nc.dram_tensor("out", [2, NPC], F32, kind="ExternalOutput")
    dbg = {}
    if DEBUG:
        for nm, shape, dt in (
                ("C", [128, NCG * 2048], BF16), ("xcatT", [128, NCG * 1536], BF16),
                ("gpT", [128, 48], F32),
                ("scols", [128, 64], F32), ("thr", [1, 32], F32),
                ("mask", [16, N], F32), ("qrow", [16, N], F32),
                ("hp", [128, NCG * 512], BF16), ("pvTb", [128, 48], BF16)):
            dbg[nm] = nc.dram_tensor("dbg_" + nm, shape, dt,
                                     kind="ExternalOutput")
    with tile.TileContext(nc) as tc:
        _emit(nc, tc, tin, t_out, dbg)
    nc.compile()
    return nc


def _emit(nc, tc, tin, t_out, dbg):
    import contextlib
    ctx = contextlib.ExitStack()
    OP = mybir.AluOpType
    ACT = mybir.ActivationFunctionType

    const = ctx.enter_context(tc.tile_pool(name="const", bufs=1))
    rows = ctx.enter_context(tc.tile_pool(name="rows", bufs=1))
    work = ctx.enter_context(tc.tile_pool(name="work", bufs=4))
    scr = ctx.enter_context(tc.tile_pool(name="scr", bufs=3))
    psmm = ctx.enter_context(tc.tile_pool(name="psmm", bufs=4, space="PSUM"))
    pspt = ctx.enter_context(tc.tile_pool(name="pspt", bufs=2, space="PSUM"))
    pscol = ctx.enter_context(tc.tile_pool(name="pscol", bufs=1, space="PSUM"))
    psaux = ctx.enter_context(tc.tile_pool(name="psaux", bufs=1,
                                           space="PSUM"))

    wb = const.tile([128, WF_TOT], BF16, tag="wb")
    bp = const.tile([128, BF_TOT], F32, tag="bp")
    xTb = const.tile([128, NCG * N], BF16, tag="xTb")   # reused as hp later
    Call = const.tile([128, NCG * 2048], BF16, tag="Call")
    xcatT = const.tile([128, NCG * 1536], BF16, tag="xcatT")

    def W(nm):
        o, w = WOFF[nm]
        return wb[:, o:o + w]

    def Bc(nm):
        o, w = BOFF[nm]
        return bp[:, o:o + w]

    nc.sync.dma_start(bp[:], tin["bpack"].ap())
    # first two C chunks before the big weight/feature loads: the fold
    # pipeline (DVE) is the head-zone bottleneck and starts on chunk 0
    nc.sync.dma_start(Call[:, 0:4096], tin["cdense"].ap()[:, 0:4096])
    nc.gpsimd.dma_start(wb[:], tin["wpack"].ap())
    nc.scalar.dma_start(xTb[:], tin["xT"].ap())
    # remaining C chunks arrive per-2cg inside the fold/layer-1 loop below

    identf = Bc("identf")
    id16 = identf[0:16, 0:16]
    id1 = identf[0:1, 0:1]
    onescol = W("ones")[:, 0:1]

    def csel(cg):
        o, _ = WOFF["csel"]
        return wb[:, o + cg * 16: o + (cg + 1) * 16]

    def rself(cg):
        o, _ = WOFF["rsel"]
        return wb[0:16, o + cg * 128: o + (cg + 1) * 128]

    # ---- degree norm rows/cols --------------------------------------------
    deg_rows = Bc("deg")[0:16, :]
    sq_row = rows.tile([16, N], F32, tag="sq")
    nc.scalar.activation(sq_row[:], deg_rows, ACT.Sqrt)
    rsd_row = rows.tile([16, N], F32, tag="rsd")
    nc.vector.reciprocal(rsd_row[:], sq_row[:])
    rsd_rowb = rows.tile([16, N], BF16, tag="rsdb")
    nc.vector.tensor_copy(rsd_rowb[:], rsd_row[:])

    rsdcol = const.tile([128, 64], F32, tag="rsdcol")
    sqcol = const.tile([128, 64], F32, tag="sqcol")
    nc.scalar.activation(sqcol[:], Bc("degcol"), ACT.Sqrt)
    nc.vector.reciprocal(rsdcol[:], sqcol[:])

    # ---- fold norm into C + layer 1, pipelined behind chunked C DMA -------
    meanT = const.tile([128, 48], F32, tag="meanT")
    meanT2 = const.tile([128, 48], F32, tag="meanT2")
    zeros256 = const.tile([128, 256], BF16, tag="zeros256")
    nc.vector.memset(zeros256[:], 0.0)

    def layer(l, cg):
        wl = W(("W1", "W2", "W3")[l])
        bcol = Bc("bcols")[:, l:l + 1]
        pxw = psmm.tile([128, 512], F32, tag="mm")
        for nt in range(4):
            if l == 0:
                lhsT = xTb[:, cg * N + nt * 128: cg * N + (nt + 1) * 128]
            else:
                lhsT = xcatT[:, cg * 1536 + (l - 1) * 512 + nt * 128:
                             cg * 1536 + (l - 1) * 512 + (nt + 1) * 128]
            nc.tensor.matmul(pxw[:, nt * 128:(nt + 1) * 128], lhsT=lhsT,
                             rhs=wl, start=True, stop=True)
        xws = scr.tile([128, 512], BF16, tag="xws")
        if l == 0:
            nc.scalar.activation(xws[:], pxw[:], ACT.Copy)
        else:
            nc.vector.tensor_copy(xws[:, 0:256], pxw[:, 0:256])
            nc.scalar.activation(xws[:, 256:512], pxw[:, 256:512], ACT.Copy)
        ph = psmm.tile([128, 512], F32, tag="mm")
        for sblk in range(4):
            nc.tensor.matmul(
                ph[:],
                lhsT=xws[:, sblk * 128:(sblk + 1) * 128],
                rhs=Call[:, cg * 2048 + sblk * 512:
                         cg * 2048 + (sblk + 1) * 512],
                start=(sblk == 0), stop=(sblk == 3))
        out_sl = xcatT[:, cg * 1536 + l * 512: cg * 1536 + (l + 1) * 512]
        if l == 0:
            nc.scalar.activation(
                out_sl, ph[:], ACT.Relu, bias=bcol,
                accum_out=meanT[:, l * 16 + cg: l * 16 + cg + 1])
            nc.vector.memset(meanT2[:, l * 16 + cg: l * 16 + cg + 1], 0.0)
        else:
            nc.scalar.activation(
                out_sl[:, 0:256], ph[:, 0:256], ACT.Relu, bias=bcol,
                accum_out=meanT[:, l * 16 + cg: l * 16 + cg + 1])
            nc.vector.scalar_tensor_tensor(
                out_sl[:, 256:512], ph[:, 256:512], bcol, zeros256[:],
                op0=OP.add, op1=OP.max,
                accum_out=meanT2[:, l * 16 + cg: l * 16 + cg + 1])

    def fold_l0(cg):
        if cg % 2 == 0 and cg >= 2:
            nc.sync.dma_start(Call[:, cg * 2048:(cg + 2) * 2048],
                              tin["cdense"].ap()[:, cg * 2048:(cg + 2) * 2048])
        pbps = psmm.tile([128, N], F32, tag="mm")
        nc.tensor.matmul(pbps[:], lhsT=rself(cg), rhs=rsd_rowb[:],
                         start=True, stop=True)
        for sblk in range(4):
            sl = Call[:, cg * 2048 + sblk * 512: cg * 2048 + (sblk + 1) * 512]
            nc.vector.scalar_tensor_tensor(
                sl, sl, rsdcol[:, sblk * 16 + cg: sblk * 16 + cg + 1],
                pbps[:], op0=OP.mult, op1=OP.mult)
        layer(0, cg)

    # layers 2+3 interleaved (software-pipelined); per-cg after layer 3:
    # c = tanh(mean @ Wg), alpha cols, sigmoid, node-major xcat copy via
    # DMA transpose, then gp via N=1 matmuls
    meanTb = rows.tile([128, 48], BF16, tag="meanTb")
    msum = rows.tile([128, 48], F32, tag="msum")
    cTb = rows.tile([128, 48], BF16, tag="cTb")
    asigb = rows.tile([128, 64], BF16, tag="asigb")
    pca = pscol.tile([128, 64], F32, tag="cols")
    gpTp = psaux.tile([128, 64], F32, tag="aux")

    def attn(cg):
        nc.vector.tensor_tensor(msum[:, cg::16], meanT[:, cg::16],
                                meanT2[:, cg::16], op=OP.add)
        nc.vector.tensor_scalar(meanTb[:, cg::16], msum[:, cg::16],
                                1.0 / N, None, op0=OP.mult)
        pc = pspt.tile([128, 128], F32, tag="pt")
        for fo in range(3):
            for fi in range(3):
                nc.tensor.matmul(
                    pc[:, fo:fo + 1],
                    lhsT=W(f"Wg{fi}")[:, fo * 128:(fo + 1) * 128],
                    rhs=meanTb[:, fi * 16 + cg: fi * 16 + cg + 1],
                    start=(fi == 0), stop=(fi == 2))
        nc.scalar.activation(cTb[:, cg::16], pc[:, 0:3], ACT.Tanh)
        for nt in range(4):
            for ch in range(3):
                nc.tensor.matmul(
                    pca[:, nt * 16 + cg: nt * 16 + cg + 1],
                    lhsT=xcatT[:, cg * 1536 + ch * 512 + nt * 128:
                               cg * 1536 + ch * 512 + (nt + 1) * 128],
                    rhs=cTb[:, ch * 16 + cg: ch * 16 + cg + 1],
                    start=(ch == 0), stop=(ch == 2))
        nc.scalar.activation(asigb[:, cg::16], pca[:, cg::16], ACT.Sigmoid)
        xcN = scr.tile([128, 1536], BF16, tag="xcN")
        nc.sync.dma_start_transpose(
            xcN[:].rearrange("p (c f) -> p c f", c=12, f=128),
            xcatT[:, cg * 1536:(cg + 1) * 1536])
        for ch in range(3):
            for nt in range(4):
                nc.tensor.matmul(
                    gpTp[:, ch * 16 + cg: ch * 16 + cg + 1],
                    lhsT=xcN[:, (ch * 4 + nt) * 128:(ch * 4 + nt + 1) * 128],
                    rhs=asigb[:, nt * 16 + cg: nt * 16 + cg + 1],
                    start=(nt == 0), stop=(nt == 3))

    for i in range(NCG + 4):
        if i < NCG:
            fold_l0(i)
        if 2 <= i < NCG + 2:
            layer(1, i - 2)
        if i >= 4:
            layer(2, i - 4)
            attn(i - 4)
    if DEBUG:
        nc.sync.dma_start(dbg["C"].ap(), Call[:])
    if DEBUG:
        nc.sync.dma_start(dbg["xcatT"].ap(), xcatT[:])
        nc.sync.dma_start(dbg["gpT"].ap(), gpTp[:, 0:48])
    gpT = gpTp

    # ---- pv = att_lin(concat(gp1, gp2)) -----------------------------------
    gpcatTb = rows.tile([128, 48], BF16, tag="gpcatTb")
    for j in range(6):
        comp, ch = j // 3, j % 3
        nc.vector.tensor_copy(
            gpcatTb[:, j * 8:(j + 1) * 8],
            gpT[:, ch * 16 + comp * 8: ch * 16 + comp * 8 + 8])
    pvTb = rows.tile([128, 48], BF16, tag="pvTb")
    for co in range(6):
        pp = pspt.tile([128, 128], F32, tag="pt")
        for ci in range(6):
            nc.tensor.matmul(pp[:, 0:8],
                             lhsT=W(f"Wal{ci}")[:, co * 128:(co + 1) * 128],
                             rhs=gpcatTb[:, ci * 8:(ci + 1) * 8],
                             start=(ci == 0), stop=(ci == 5))
        nc.vector.tensor_scalar(pvTb[:, co * 8:(co + 1) * 8], pp[:, 0:8],
                                Bc("balcol")[:, co:co + 1], None, op0=OP.add)
    if DEBUG:
        nc.sync.dma_start(dbg["pvTb"].ap(), pvTb[:])

    # ---- 1/||pv|| per graph ------------------------------------------------
    pnn = pspt.tile([128, 128], F32, tag="pt")
    for j in range(16):
        comp, gl = j // 8, j % 8
        for ci in range(3):
            col = pvTb[:, (comp * 3 + ci) * 8 + gl: (comp * 3 + ci) * 8 + gl + 1]
            nc.tensor.matmul(pnn[0:1, j:j + 1], lhsT=col, rhs=col,
                             start=(ci == 0), stop=(ci == 2))
    nnrow = rows.tile([1, 16], F32, tag="nnrow")
    nc.vector.tensor_copy(nnrow[:], pnn[0:1, 0:16])
    sqnrow = rows.tile([1, 16], F32, tag="sqnrow")
    nc.scalar.activation(sqnrow[:], nnrow[:], ACT.Sqrt)
    rsnrow = rows.tile([1, 16], F32, tag="rsnrow")
    nc.vector.reciprocal(rsnrow[:], sqnrow[:])
    ptn = pspt.tile([128, 128], F32, tag="pt")
    nc.tensor.transpose(ptn[0:16, 0:1], rsnrow[:], id1)
    rsncol = rows.tile([16, 1], F32, tag="rsncol")
    nc.vector.tensor_copy(rsncol[:], ptn[0:16, 0:1])

    # ---- scores (cg-major cols for kth_largest) ---------------------------
    pcs = pscol.tile([128, 64], F32, tag="cols")
    for cg in range(NCG):
        comp, gl = cg // NPC, cg % NPC
        for nt in range(4):
            for ci in range(3):
                nc.tensor.matmul(
                    pcs[:, cg * 4 + nt: cg * 4 + nt + 1],
                    lhsT=xcatT[:, cg * 1536 + ci * 512 + nt * 128:
                               cg * 1536 + ci * 512 + (nt + 1) * 128],
                    rhs=pvTb[:, (comp * 3 + ci) * 8 + gl:
                             (comp * 3 + ci) * 8 + gl + 1],
                    start=(ci == 0), stop=(ci == 2))
    scols = rows.tile([128, 64], F32, tag="scols")
    for h in range(4):
        nc.vector.tensor_copy(scols[:, h * 16:(h + 1) * 16],
                              pcs[:, h * 16:(h + 1) * 16])
    if DEBUG:
        nc.sync.dma_start(dbg["scols"].ap(), scols[:])

    thr = rows.tile([1, 32], F32, tag="thr")
    for g in range(16):
        nc.gpsimd.kth_largest(thr[0:1, 2 * g:2 * g + 2],
                              scols[:, g * 4:(g + 1) * 4],
                              n_per_lane=4, k=256, quantile=0.5005)
    if DEBUG:
        nc.sync.dma_start(dbg["thr"].ap(), thr[:])
    ptt = pspt.tile([128, 128], F32, tag="pt")
    nc.tensor.transpose(ptt[0:16, 0:1], thr[0:1, 0::2], id1)
    thrcol = rows.tile([16, 1], F32, tag="thrcol")
    nc.vector.tensor_copy(thrcol[:], ptt[0:16, 0:1])

    score_row = rows.tile([16, N], F32, tag="score")
    sig_row = rows.tile([16, N], F32, tag="sig")
    for nt in range(4):
        pt = pspt.tile([128, 128], F32, tag="pt")
        nc.tensor.transpose(pt[0:16, :], scols[:, nt::4], identf)
        nc.vector.tensor_copy(score_row[:, nt * 128:(nt + 1) * 128],
                              pt[0:16, :])
        nc.scalar.activation(sig_row[:, nt * 128:(nt + 1) * 128],
                             pt[0:16, :], ACT.Sigmoid, scale=rsncol[:])

    mask_row = rows.tile([16, N], F32, tag="mask")
    nc.vector.tensor_scalar(mask_row[:], score_row[:], thrcol[:], None,
                            op0=OP.is_gt)
    if DEBUG:
        nc.sync.dma_start(dbg["mask"].ap(), mask_row[:])

    # ---- pooled degree -----------------------------------------------------
    mcol = const.tile([128, 64], F32, tag="mcol")
    for sblk in range(4):
        pt = pspt.tile([128, 128], F32, tag="pt")
        nc.tensor.transpose(pt[:, 0:16],
                            mask_row[:, sblk * 128:(sblk + 1) * 128], id16)
        nc.vector.tensor_copy(mcol[:, sblk * 16:(sblk + 1) * 16], pt[:, 0:16])
    msqcol = const.tile([128, 64], F32, tag="msqcol")
    nc.vector.tensor_tensor(msqcol[:], mcol[:], sqcol[:], op=OP.mult)

    ps_d2 = psaux.tile([16, N], F32, tag="aux")
    for cg in range(NCG):
        for sblk in range(4):
            mlh = work.tile([128, 16], BF16, tag="mlh")
            nc.vector.tensor_scalar(
                mlh[:], csel(cg),
                msqcol[:, sblk * 16 + cg: sblk * 16 + cg + 1], None,
                op0=OP.mult)
            nc.tensor.matmul(
                ps_d2[:], lhsT=mlh[:],
                rhs=Call[:, cg * 2048 + sblk * 512: cg * 2048 + (sblk + 1) * 512],
                start=(cg == 0 and sblk == 0),
                stop=(cg == NCG - 1 and sblk == 3))
    sqm_row = rows.tile([16, N], F32, tag="sqm")
    nc.vector.tensor_tensor(sqm_row[:], sq_row[:], mask_row[:], op=OP.mult)
    d2a = rows.tile([16, N], F32, tag="d2a")
    nc.vector.tensor_tensor(d2a[:], ps_d2[:], sqm_row[:], op=OP.mult)
    d2b = rows.tile([16, N], F32, tag="d2b")
    nc.vector.tensor_tensor(d2b[:], d2a[:], mask_row[:], op=OP.subtract)
    sq2_row = rows.tile([16, N], F32, tag="sq2")
    nc.scalar.activation(sq2_row[:], d2b[:], ACT.Sqrt, bias=1.0)
    rsd2_row = rows.tile([16, N], F32, tag="rsd2")
    nc.vector.reciprocal(rsd2_row[:], sq2_row[:])
    mr2_row = rows.tile([16, N], F32, tag="mr2")
    nc.vector.tensor_tensor(mr2_row[:], rsd2_row[:], mask_row[:], op=OP.mult)
    q_row = rows.tile([16, N], F32, tag="qrow")
    nc.vector.tensor_tensor(q_row[:], mr2_row[:], sq_row[:], op=OP.mult)
    gate2_row = rows.tile([16, N], F32, tag="gate2")
    nc.vector.tensor_tensor(gate2_row[:], sig_row[:], q_row[:], op=OP.mult)
    if DEBUG:
        nc.sync.dma_start(dbg["qrow"].ap(), q_row[:])

    qcol = const.tile([128, 64], F32, tag="qcol")
    g2col = const.tile([128, 64], F32, tag="g2col")
    for sblk in range(4):
        pt = pspt.tile([128, 128], F32, tag="pt")
        nc.tensor.transpose(pt[:, 0:16],
                            q_row[:, sblk * 128:(sblk + 1) * 128], id16)
        nc.vector.tensor_copy(qcol[:, sblk * 16:(sblk + 1) * 16], pt[:, 0:16])
        pt2 = pspt.tile([128, 128], F32, tag="pt")
        nc.tensor.transpose(pt2[:, 0:16],
                            gate2_row[:, sblk * 128:(sblk + 1) * 128], id16)
        nc.vector.tensor_copy(g2col[:, sblk * 16:(sblk + 1) * 16],
                              pt2[:, 0:16])

    # ---- pooled conv (node-major) + fused final attention pool ------------
    hpall = xTb  # xTb fully consumed by layer 1
    bfr = Bc("bfr")
    ps_mg = pscol.tile([128, 64], F32, tag="cols")  # cols 0:16 mean, 16:32 g
    mT2b = rows.tile([128, 16], BF16, tag="mT2b")
    c2b = rows.tile([128, 16], BF16, tag="c2b")
    a4 = rows.tile([128, 64], BF16, tag="a4")
    for cg in range(NCG):
        pxp = psmm.tile([128, 512], F32, tag="mm")
        for nt in range(4):
            for ci in range(3):
                nc.tensor.matmul(
                    pxp[:, nt * 128:(nt + 1) * 128],
                    lhsT=xcatT[:, cg * 1536 + ci * 512 + nt * 128:
                               cg * 1536 + ci * 512 + (nt + 1) * 128],
                    rhs=W(f"Wf{ci}"), start=(ci == 0), stop=(ci == 2))
        xwps = scr.tile([128, 512], BF16, tag="xwps")
        for nt in range(4):
            nc.scalar.activation(
                xwps[:, nt * 128:(nt + 1) * 128],
                pxp[:, nt * 128:(nt + 1) * 128], ACT.Copy,
                scale=g2col[:, nt * 16 + cg: nt * 16 + cg + 1])
        pm = psmm.tile([128, 512], F32, tag="mm")
        for dt in range(4):
            for sblk in range(4):
                nc.tensor.matmul(
                    pm[:, dt * 128:(dt + 1) * 128],
                    lhsT=Call[:, cg * 2048 + sblk * 512 + dt * 128:
                              cg * 2048 + sblk * 512 + (dt + 1) * 128],
                    rhs=xwps[:, sblk * 128:(sblk + 1) * 128],
                    start=(sblk == 0), stop=(sblk == 3))
        hp = hpall[:, cg * 512:(cg + 1) * 512]
        y2 = scr.tile([128, 512], BF16, tag="y2")
        for dt in range(4):
            nc.vector.scalar_tensor_tensor(
                y2[:, dt * 128:(dt + 1) * 128],
                pm[:, dt * 128:(dt + 1) * 128],
                qcol[:, dt * 16 + cg: dt * 16 + cg + 1], bfr,
                op0=OP.mult, op1=OP.add)
            nc.vector.tensor_scalar(
                hp[:, dt * 128:(dt + 1) * 128],
                y2[:, dt * 128:(dt + 1) * 128], 0.0,
                mcol[:, dt * 16 + cg: dt * 16 + cg + 1],
                op0=OP.max, op1=OP.mult)
        for dt in range(4):
            nc.tensor.matmul(ps_mg[:, cg:cg + 1],
                             lhsT=hp[:, dt * 128:(dt + 1) * 128],
                             rhs=onescol, start=(dt == 0), stop=(dt == 3))
        # per-graph c2 = tanh(mean @ Wg_fin); alpha2 via transposed hp
        pt2 = pspt.tile([128, 128], F32, tag="pt")
        nc.vector.tensor_scalar(mT2b[:, cg:cg + 1], ps_mg[:, cg:cg + 1],
                                1.0 / K1, None, op0=OP.mult)
        nc.tensor.matmul(pt2[:, 4:5], lhsT=W("Wgf"), rhs=mT2b[:, cg:cg + 1],
                         start=True, stop=True)
        nc.scalar.activation(c2b[:, cg:cg + 1], pt2[:, 4:5], ACT.Tanh)
        hpT = scr.tile([128, 512], BF16, tag="hpT")
        nc.sync.dma_start_transpose(
            hpT[:].rearrange("p (c f) -> p c f", c=4, f=128), hp[:])
        for dt in range(4):
            nc.tensor.matmul(pt2[:, dt:dt + 1],
                             lhsT=hpT[:, dt * 128:(dt + 1) * 128],
                             rhs=c2b[:, cg:cg + 1], start=True, stop=True)
        nc.scalar.activation(a4[:, cg * 4:(cg + 1) * 4], pt2[:, 0:4],
                             ACT.Sigmoid)
        for dt in range(4):
            nc.tensor.matmul(ps_mg[:, 16 + cg: 16 + cg + 1],
                             lhsT=hp[:, dt * 128:(dt + 1) * 128],
                             rhs=a4[:, cg * 4 + dt: cg * 4 + dt + 1],
                             start=(dt == 0), stop=(dt == 3))
    if DEBUG:
        nc.sync.dma_start(dbg["hp"].ap(), hpall[:])

    # ---- head MLP ----------------------------------------------------------
    pcat = rows.tile([128, 16], BF16, tag="pcat")
    nc.vector.tensor_copy(pcat[:], ps_mg[:, 16:32])
    p1 = pspt.tile([128, 128], F32, tag="pt")
    nc.tensor.matmul(p1[:, 0:NPC], lhsT=W("Wl1a"), rhs=pcat[:, 0:NPC],
                     start=True, stop=False)
    nc.tensor.matmul(p1[:, 0:NPC], lhsT=W("Wl1b"), rhs=pcat[:, NPC:2 * NPC],
                     start=False, stop=True)
    o1 = rows.tile([128, NPC], BF16, tag="o1")
    nc.scalar.activation(o1[:], p1[:, 0:NPC], ACT.Relu, bias=Bc("bl1col")[:])
    p2 = pspt.tile([128, 128], F32, tag="pt")
    nc.tensor.matmul(p2[0:64, 0:NPC], lhsT=W("Wl2"), rhs=o1[:], start=True,
                     stop=True)
    o2 = rows.tile([64, NPC], BF16, tag="o2")
    nc.scalar.activation(o2[:], p2[0:64, 0:NPC], ACT.Relu,
                         bias=Bc("bl2col")[0:64, :])
    p3 = pspt.tile([128, 128], F32, tag="pt")
    nc.tensor.matmul(p3[0:2, 0:NPC], lhsT=W("Wl3")[0:64, :], rhs=o2[:],
                     start=True, stop=True)
    o3 = rows.tile([2, NPC], F32, tag="o3")
    nc.vector.tensor_scalar(o3[:], p3[0:2, 0:NPC], Bc("bl3col")[0:2, :],
                            None, op0=OP.add)
    nc.sync.dma_start(t_out.ap(), o3[:])
    ctx.close()


_NC_CACHE = {}


def _get_nc():
    key = (DEBUG,)
    if key not in _NC_CACHE:
        _NC_CACHE[key] = _build()
    return _NC_CACHE[key]


def kernel(**inputs):
    in_maps = _host_prep(inputs)
    nc = _get_nc()
    res = run_bass_kernel_spmd(nc, in_maps, core_ids=list(range(NCORES)))
    out = np.empty((B, 2), np.float32)
    for c in range(NCORES):
        out[c * NPC:(c + 1) * NPC] = res.results[c]["out"].T
    kernel._last = res
    kernel._nc = nc
    return out


# revision 70
# speedup vs baseline: 1.0040x; 1.0037x over previous
"""Trainium2 Bass kernel for nn_CAGpool (GNN message passing, CAG pooling).

Sharding: data-parallel over the 64 graph pairs -> 8 pairs (16 component
graphs of 512 nodes) per NeuronCore.  Message passing is dense matmul
against a per-graph 512x512 adjacency DMA'd from host as raw edge counts
(A+I); degree rows likewise host-derived from the integer edge lists.
Symmetric norm folded into C on-device with fused scalar_tensor_tensor.

Reductions over features use N=1 matmul columns; reductions over nodes use
stt-accum ops; the top-256 threshold comes from gpsimd kth_largest.
"""

import os
import numpy as np
import ml_dtypes

import concourse.bass as bass
import concourse.tile as tile
from concourse import bacc, mybir
from concourse.bass_utils import run_bass_kernel_spmd

F32 = mybir.dt.float32
BF16 = mybir.dt.bfloat16

NCORES = 8
B = 64
NPC = B // NCORES          # graph pairs per core (8)
NCG = 2 * NPC              # component graphs per core (16)
N = 512                    # nodes per component graph
K1 = 256
DEBUG = bool(int(os.environ.get("KERNEL_DEBUG", "0")))



def _layout(ent):
    offs, off = {}, 0
    for nm, w in ent:
        offs[nm] = (off, w)
        off += w
    return offs, off


WOFF, WF_TOT = _layout(
    [("W1", 128), ("W2", 128), ("W3", 128), ("Wgf", 128)]
    + [(f"Wg{i}", 384) for i in range(3)]
    + [(f"Wal{i}", 768) for i in range(6)]
    + [(f"Wf{i}", 128) for i in range(3)]
    + [("Wl1a", 128), ("Wl1b", 128), ("Wl2", 64), ("Wl3", 2),
       ("ones", 128), ("csel", 256), ("rsel", 2048)])
BOFF, BF_TOT = _layout(
    [("bfr", 128), ("balcol", 6), ("bl1col", 1), ("bl2col", 1),
     ("bl3col", 1), ("identf", 128), ("bcols", 3), ("deg", 512),
     ("degcol", 64)])


def _host_prep(inputs):
    """Per-core input maps. Index-structure prep only: dense adjacency counts
    and degree counts come straight from the integer edge lists."""
    x = np.asarray(inputs["x"], np.float32)

    s_loc, d_loc = {}, {}
    for comp, (sk, dk) in enumerate((("src_c1", "dst_c1"),
                                     ("src_c2", "dst_c2"))):
        base = (np.arange(B) * N)[:, None]
        s_loc[comp] = (np.asarray(inputs[sk]).reshape(B, -1) - base).astype(np.int64)
        d_loc[comp] = (np.asarray(inputs[dk]).reshape(B, -1) - base).astype(np.int64)

    in_maps = []
    loop = np.arange(N, dtype=np.int64)
    for c in range(NCORES):
        xT = np.empty((128, NCG * N), np.float32)
        deg = np.zeros((16, N), np.float32)
        idx_parts = []
        for comp in range(2):
            for gl in range(NPC):
                g = c * NPC + gl
                cg = comp * NPC + gl
                r0 = g * 2 * N + comp * N
                xT[:, cg * N:(cg + 1) * N] = x[r0:r0 + N].T
                s = np.concatenate([s_loc[comp][g], loop])
                d = np.concatenate([d_loc[comp][g], loop])
                idx = (s & 127) * (NCG * 2048) + cg * 2048 + (s >> 7) * 512 + d
                idx_parts.append(idx)
                deg[cg] = np.bincount(d_loc[comp][g], minlength=N) + 1.0
        cnt = np.bincount(np.concatenate(idx_parts),
                          minlength=128 * NCG * 2048)
        cdense = cnt.astype(ml_dtypes.bfloat16).reshape(128, NCG * 2048)

        wpack = np.zeros((128, WF_TOT), np.float32)

        def put(nm, arr):
            o, w = WOFF[nm]
            arr = np.asarray(arr, np.float32)
            wpack[: arr.shape[0], o:o + arr.shape[1]] = arr

        put("W1", inputs["W1"]); put("W2", inputs["W2"]); put("W3", inputs["W3"])
        put("Wgf", inputs["Wg_fin"])
        for i in range(3):
            put(f"Wg{i}", np.asarray(inputs["Wg_att"])[i * 128:(i + 1) * 128])
        for i in range(6):
            put(f"Wal{i}", np.asarray(inputs["Wal"])[i * 128:(i + 1) * 128])
        for i in range(3):
            put(f"Wf{i}", np.asarray(inputs["Wf"])[i * 128:(i + 1) * 128])
        put("Wl1a", np.asarray(inputs["Wl1"])[:128])
        put("Wl1b", np.asarray(inputs["Wl1"])[128:])
        put("Wl2", inputs["Wl2"])
        put("Wl3", inputs["Wl3"])
        put("ones", np.ones((128, 128), np.float32))
        csel = np.zeros((128, 256), np.float32)
        for cg in range(NCG):
            csel[:, cg * 16 + cg] = 1.0
        put("csel", csel)
        rsel = np.zeros((16, 2048), np.float32)
        for cg in range(16):
            rsel[cg, cg * 128:(cg + 1) * 128] = 1.0
        put("rsel", rsel)

        bpack = np.zeros((128, BF_TOT), np.float32)

        def putb(nm, arr):
            o, w = BOFF[nm]
            arr = np.asarray(arr, np.float32)
            bpack[: arr.shape[0], o:o + arr.shape[1]] = arr

        putb("bfr", np.broadcast_to(np.asarray(inputs["bf"])[None, :],
                                    (128, 128)))
        putb("balcol", np.asarray(inputs["bal"]).reshape(6, 128).T)
        putb("bl1col", np.asarray(inputs["bl1"])[:, None])
        putb("bl2col", np.asarray(inputs["bl2"])[:, None])
        putb("bl3col", np.asarray(inputs["bl3"])[:, None])
        putb("bcols", np.stack([np.asarray(inputs["b1"]),
                                np.asarray(inputs["b2"]),
                                np.asarray(inputs["b3"])], 1))
        putb("identf", np.eye(128, dtype=np.float32))
        putb("deg", deg)
        degcol = np.empty((128, 64), np.float32)
        for sblk in range(4):
            degcol[:, sblk * 16:(sblk + 1) * 16] = \
                deg[:, sblk * 128:(sblk + 1) * 128].T
        putb("degcol", degcol)

        in_maps.append({"xT": np.ascontiguousarray(xT.astype(ml_dtypes.bfloat16)),
                        "cdense": np.ascontiguousarray(cdense),
                        "wpack": wpack.astype(ml_dtypes.bfloat16),
                        "bpack": bpack})
    return in_maps


def _build():
    nc = bacc.Bacc("TRN2", target_bir_lowering=False, debug=False,
                   num_devices=NCORES)
    tin = {
        "xT": nc.dram_tensor("xT", [128, NCG * N], BF16, kind="ExternalInput"),
        "cdense": nc.dram_tensor("cdense", [128, NCG * 2048], BF16,
                                 kind="ExternalInput"),
        "wpack": nc.dram_tensor("wpack", [128, WF_TOT], BF16,
                                kind="ExternalInput"),
        "bpack": nc.dram_tensor("bpack", [128, BF_TOT], F32,
                                kind="ExternalInput"),
    }
    t_out = nc.dram_tensor("out", [2, NPC], F32, kind="ExternalOutput")
    dbg = {}
    if DEBUG:
        for nm, shape, dt in (
                ("C", [128, NCG * 2048], BF16), ("xcatT", [128, NCG * 1536], BF16),
                ("gpT", [128, 48], F32),
                ("scols", [128, 64], F32), ("thr", [1, 32], F32),
                ("mask", [16, N], F32), ("qrow", [16, N], F32),
                ("hp", [128, NCG * 512], BF16), ("pvTb", [128, 48], BF16)):
            dbg[nm] = nc.dram_tensor("dbg_" + nm, shape, dt,
                                     kind="ExternalOutput")
    with tile.TileContext(nc) as tc:
        _emit(nc, tc, tin, t_out, dbg)
    nc.compile()
    return nc


def _emit(nc, tc, tin, t_out, dbg):
    import contextlib
    ctx = contextlib.ExitStack()
    OP = mybir.AluOpType
    ACT = mybir.ActivationFunctionType

    const = ctx.enter_context(tc.tile_pool(name="const", bufs=1))
    rows = ctx.enter_context(tc.tile_pool(name="rows", bufs=1))
    work = ctx.enter_context(tc.tile_pool(name="work", bufs=4))
    scr = ctx.enter_context(tc.tile_pool(name="scr", bufs=3))
    psmm = ctx.enter_context(tc.tile_pool(name="psmm", bufs=4, space="PSUM"))
    pspt = ctx.enter_context(tc.tile_pool(name="pspt", bufs=2, space="PSUM"))
    pscol = ctx.enter_context(tc.tile_pool(name="pscol", bufs=1, space="PSUM"))
    psaux = ctx.enter_context(tc.tile_pool(name="psaux", bufs=1,
                                           space="PSUM"))

    wb = const.tile([128, WF_TOT], BF16, tag="wb")
    bp = const.tile([128, BF_TOT], F32, tag="bp")
    xTb = const.tile([128, NCG * N], BF16, tag="xTb")   # reused as hp later
    Call = const.tile([128, NCG * 2048], BF16, tag="Call")
    xcatT = const.tile([128, NCG * 1536], BF16, tag="xcatT")

    def W(nm):
        o, w = WOFF[nm]
        return wb[:, o:o + w]

    def Bc(nm):
        o, w = BOFF[nm]
        return bp[:, o:o + w]

    nc.sync.dma_start(bp[:], tin["bpack"].ap())
    # first two C chunks before the big weight/feature loads: the fold
    # pipeline (DVE) is the head-zone bottleneck and starts on chunk 0
    nc.sync.dma_start(Call[:, 0:4096], tin["cdense"].ap()[:, 0:4096])
    nc.gpsimd.dma_start(wb[:], tin["wpack"].ap())
    nc.scalar.dma_start(xTb[:], tin["xT"].ap())
    # remaining C chunks arrive per-2cg inside the fold/layer-1 loop below

    identf = Bc("identf")
    id16 = identf[0:16, 0:16]
    id1 = identf[0:1, 0:1]
    onescol = W("ones")[:, 0:1]

    def csel(cg):
        o, _ = WOFF["csel"]
        return wb[:, o + cg * 16: o + (cg + 1) * 16]

    def rself(cg):
        o, _ = WOFF["rsel"]
        return wb[0:16, o + cg * 128: o + (cg + 1) * 128]

    # ---- degree norm rows/cols --------------------------------------------
    deg_rows = Bc("deg")[0:16, :]
    sq_row = rows.tile([16, N], F32, tag="sq")
    nc.scalar.activation(sq_row[:], deg_rows, ACT.Sqrt)
    rsd_row = rows.tile([16, N], F32, tag="rsd")
    nc.vector.reciprocal(rsd_row[:], sq_row[:])
    rsd_rowb = rows.tile([16, N], BF16, tag="rsdb")
    nc.vector.tensor_copy(rsd_rowb[:], rsd_row[:])

    rsdcol = const.tile([128, 64], F32, tag="rsdcol")
    sqcol = const.tile([128, 64], F32, tag="sqcol")
    nc.scalar.activation(sqcol[:], Bc("degcol"), ACT.Sqrt)
    nc.vector.reciprocal(rsdcol[:], sqcol[:])

    # ---- fold norm into C + layer 1, pipelined behind chunked C DMA -------
    meanT = const.tile([128, 48], F32, tag="meanT")
    meanT2 = const.tile([128, 48], F32, tag="meanT2")
    zeros256 = const.tile([128, 256], BF16, tag="zeros256")
    nc.vector.memset(zeros256[:], 0.0)

    def layer(l, cg):
        wl = W(("W1", "W2", "W3")[l])
        bcol = Bc("bcols")[:, l:l + 1]
        pxw = psmm.tile([128, 512], F32, tag="mm")
        for nt in range(4):
            if l == 0:
                lhsT = xTb[:, cg * N + nt * 128: cg * N + (nt + 1) * 128]
            else:
                lhsT = xcatT[:, cg * 1536 + (l - 1) * 512 + nt * 128:
                             cg * 1536 + (l - 1) * 512 + (nt + 1) * 128]
            nc.tensor.matmul(pxw[:, nt * 128:(nt + 1) * 128], lhsT=lhsT,
                             rhs=wl, start=True, stop=True)
        xws = scr.tile([128, 512], BF16, tag="xws")
        if l == 0:
            nc.scalar.activation(xws[:], pxw[:], ACT.Copy)
        else:
            nc.vector.tensor_copy(xws[:, 0:256], pxw[:, 0:256])
            nc.scalar.activation(xws[:, 256:512], pxw[:, 256:512], ACT.Copy)
        ph = psmm.tile([128, 512], F32, tag="mm")
        for sblk in range(4):
            nc.tensor.matmul(
                ph[:],
                lhsT=xws[:, sblk * 128:(sblk + 1) * 128],
                rhs=Call[:, cg * 2048 + sblk * 512:
                         cg * 2048 + (sblk + 1) * 512],
                start=(sblk == 0), stop=(sblk == 3))
        out_sl = xcatT[:, cg * 1536 + l * 512: cg * 1536 + (l + 1) * 512]
        if l == 0:
            nc.scalar.activation(
                out_sl, ph[:], ACT.Relu, bias=bcol,
                accum_out=meanT[:, l * 16 + cg: l * 16 + cg + 1])
            nc.vector.memset(meanT2[:, l * 16 + cg: l * 16 + cg + 1], 0.0)
        else:
            nc.scalar.activation(
                out_sl[:, 0:256], ph[:, 0:256], ACT.Relu, bias=bcol,
                accum_out=meanT[:, l * 16 + cg: l * 16 + cg + 1])
            nc.vector.scalar_tensor_tensor(
                out_sl[:, 256:512], ph[:, 256:512], bcol, zeros256[:],
                op0=OP.add, op1=OP.max,
                accum_out=meanT2[:, l * 16 + cg: l * 16 + cg + 1])

    def fold_l0(cg):
        if cg % 2 == 0 and cg >= 2:
            nc.sync.dma_start(Call[:, cg * 2048:(cg + 2) * 2048],
                              tin["cdense"].ap()[:, cg * 2048:(cg + 2) * 2048])
        pbps = psmm.tile([128, N], F32, tag="mm")
        nc.tensor.matmul(pbps[:], lhsT=rself(cg), rhs=rsd_rowb[:],
                         start=True, stop=True)
        for sblk in range(4):
            sl = Call[:, cg * 2048 + sblk * 512: cg * 2048 + (sblk + 1) * 512]
            nc.vector.scalar_tensor_tensor(
                sl, sl, rsdcol[:, sblk * 16 + cg: sblk * 16 + cg + 1],
                pbps[:], op0=OP.mult, op1=OP.mult)
        layer(0, cg)

    # layers 2+3 interleaved (software-pipelined); per-cg after layer 3:
    # c = tanh(mean @ Wg), alpha cols, sigmoid, node-major xcat copy via
    # DMA transpose, then gp via N=1 matmuls
    meanTb = rows.tile([128, 48], BF16, tag="meanTb")
    msum = rows.tile([128, 48], F32, tag="msum")
    cTb = rows.tile([128, 48], BF16, tag="cTb")
    asigb = rows.tile([128, 64], BF16, tag="asigb")
    pca = pscol.tile([128, 64], F32, tag="cols")
    gpTp = psaux.tile([128, 64], F32, tag="aux")

    def attn(cg):
        nc.vector.tensor_tensor(msum[:, cg::16], meanT[:, cg::16],
                                meanT2[:, cg::16], op=OP.add)
        nc.vector.tensor_scalar(meanTb[:, cg::16], msum[:, cg::16],
                                1.0 / N, None, op0=OP.mult)
        pc = pspt.tile([128, 128], F32, tag="pt")
        for fo in range(3):
            for fi in range(3):
                nc.tensor.matmul(
                    pc[:, fo:fo + 1],
                    lhsT=W(f"Wg{fi}")[:, fo * 128:(fo + 1) * 128],
                    rhs=meanTb[:, fi * 16 + cg: fi * 16 + cg + 1],
                    start=(fi == 0), stop=(fi == 2))
        nc.scalar.activation(cTb[:, cg::16], pc[:, 0:3], ACT.Tanh)
        for nt in range(4):
            for ch in range(3):
                nc.tensor.matmul(
                    pca[:, nt * 16 + cg: nt * 16 + cg + 1],
                    lhsT=xcatT[:, cg * 1536 + ch * 512 + nt * 128:
                               cg * 1536 + ch * 512 + (nt + 1) * 128],
                    rhs=cTb[:, ch * 16 + cg: ch * 16 + cg + 1],
                    start=(ch == 0), stop=(ch == 2))
        nc.scalar.activation(asigb[:, cg::16], pca[:, cg::16], ACT.Sigmoid)
        xcN = scr.tile([128, 1536], BF16, tag="xcN")
        nc.sync.dma_start_transpose(
            xcN[:].rearrange("p (c f) -> p c f", c=12, f=128),
            xcatT[:, cg * 1536:(cg + 1) * 1536])
        for ch in range(3):
            for nt in range(4):
                nc.tensor.matmul(
                    gpTp[:, ch * 16 + cg: ch * 16 + cg + 1],
                    lhsT=xcN[:, (ch * 4 + nt) * 128:(ch * 4 + nt + 1) * 128],
                    rhs=asigb[:, nt * 16 + cg: nt * 16 + cg + 1],
                    start=(nt == 0), stop=(nt == 3))

    for i in range(NCG + 4):
        if i < NCG:
            fold_l0(i)
        if 2 <= i < NCG + 2:
            layer(1, i - 2)
        if i >= 4:
            layer(2, i - 4)
            attn(i - 4)
    if DEBUG:
        nc.sync.dma_start(dbg["C"].ap(), Call[:])
    if DEBUG:
        nc.sync.dma_start(dbg["xcatT"].ap(), xcatT[:])
        nc.sync.dma_start(dbg["gpT"].ap(), gpTp[:, 0:48])
    gpT = gpTp

    # ---- pv = att_lin(concat(gp1, gp2)) -----------------------------------
    gpcatTb = rows.tile([128, 48], BF16, tag="gpcatTb")
    for j in range(6):
        comp, ch = j // 3, j % 3
        nc.vector.tensor_copy(
            gpcatTb[:, j * 8:(j + 1) * 8],
            gpT[:, ch * 16 + comp * 8: ch * 16 + comp * 8 + 8])
    pvTb = rows.tile([128, 48], BF16, tag="pvTb")
    for co in range(6):
        pp = pspt.tile([128, 128], F32, tag="pt")
        for ci in range(6):
            nc.tensor.matmul(pp[:, 0:8],
                             lhsT=W(f"Wal{ci}")[:, co * 128:(co + 1) * 128],
                             rhs=gpcatTb[:, ci * 8:(ci + 1) * 8],
                             start=(ci == 0), stop=(ci == 5))
        nc.vector.tensor_scalar(pvTb[:, co * 8:(co + 1) * 8], pp[:, 0:8],
                                Bc("balcol")[:, co:co + 1], None, op0=OP.add)
    if DEBUG:
        nc.sync.dma_start(dbg["pvTb"].ap(), pvTb[:])

    # ---- 1/||pv|| per graph ------------------------------------------------
    pnn = pspt.tile([128, 128], F32, tag="pt")
    for j in range(16):
        comp, gl = j // 8, j % 8
        for ci in range(3):
            col = pvTb[:, (comp * 3 + ci) * 8 + gl: (comp * 3 + ci) * 8 + gl + 1]
            nc.tensor.matmul(pnn[0:1, j:j + 1], lhsT=col, rhs=col,
                             start=(ci == 0), stop=(ci == 2))
    nnrow = rows.tile([1, 16], F32, tag="nnrow")
    nc.vector.tensor_copy(nnrow[:], pnn[0:1, 0:16])
    sqnrow = rows.tile([1, 16], F32, tag="sqnrow")
    nc.scalar.activation(sqnrow[:], nnrow[:], ACT.Sqrt)
    rsnrow = rows.tile([1, 16], F32, tag="rsnrow")
    nc.vector.reciprocal(rsnrow[:], sqnrow[:])
    ptn = pspt.tile([128, 128], F32, tag="pt")
    nc.tensor.transpose(ptn[0:16, 0:1], rsnrow[:], id1)
    rsncol = rows.tile([16, 1], F32, tag="rsncol")
    nc.vector.tensor_copy(rsncol[:], ptn[0:16, 0:1])

    # ---- scores (cg-major cols for kth_largest) ---------------------------
    pcs = pscol.tile([128, 64], F32, tag="cols")
    for cg in range(NCG):
        comp, gl = cg // NPC, cg % NPC
        for nt in range(4):
            for ci in range(3):
                nc.tensor.matmul(
                    pcs[:, cg * 4 + nt: cg * 4 + nt + 1],
                    lhsT=xcatT[:, cg * 1536 + ci * 512 + nt * 128:
                               cg * 1536 + ci * 512 + (nt + 1) * 128],
                    rhs=pvTb[:, (comp * 3 + ci) * 8 + gl:
                             (comp * 3 + ci) * 8 + gl + 1],
                    start=(ci == 0), stop=(ci == 2))
    scols = rows.tile([128, 64], F32, tag="scols")
    for h in range(4):
        nc.vector.tensor_copy(scols[:, h * 16:(h + 1) * 16],
                              pcs[:, h * 16:(h + 1) * 16])
    if DEBUG:
        nc.sync.dma_start(dbg["scols"].ap(), scols[:])

    thr = rows.tile([1, 32], F32, tag="thr")
    for g in range(16):
        nc.gpsimd.kth_largest(thr[0:1, 2 * g:2 * g + 2],
                              scols[:, g * 4:(g + 1) * 4],
                              n_per_lane=4, k=256, quantile=0.5005)
    if DEBUG:
        nc.sync.dma_start(dbg["thr"].ap(), thr[:])
    ptt = pspt.tile([128, 128], F32, tag="pt")
    nc.tensor.transpose(ptt[0:16, 0:1], thr[0:1, 0::2], id1)
    thrcol = rows.tile([16, 1], F32, tag="thrcol")
    nc.vector.tensor_copy(thrcol[:], ptt[0:16, 0:1])

    score_row = rows.tile([16, N], F32, tag="score")
    sig_row = rows.tile([16, N], F32, tag="sig")
    for nt in range(4):
        pt = pspt.tile([128, 128], F32, tag="pt")
        nc.tensor.transpose(pt[0:16, :], scols[:, nt::4], identf)
        nc.vector.tensor_copy(score_row[:, nt * 128:(nt + 1) * 128],
                              pt[0:16, :])
        nc.scalar.activation(sig_row[:, nt * 128:(nt + 1) * 128],
                             pt[0:16, :], ACT.Sigmoid, scale=rsncol[:])

    mask_row = rows.tile([16, N], F32, tag="mask")
    nc.vector.tensor_scalar(mask_row[:], score_row[:], thrcol[:], None,
                            op0=OP.is_gt)
    if DEBUG:
        nc.sync.dma_start(dbg["mask"].ap(), mask_row[:])

    # ---- pooled degree -----------------------------------------------------
    mcol = const.tile([128, 64], F32, tag="mcol")
    for sblk in range(4):
        pt = pspt.tile([128, 128], F32, tag="pt")
        nc.tensor.transpose(pt[:, 0:16],
                            mask_row[:, sblk * 128:(sblk + 1) * 128], id16)
        nc.vector.tensor_copy(mcol[:, sblk * 16:(sblk + 1) * 16], pt[:, 0:16])
    msqcol = const.tile([128, 64], F32, tag="msqcol")
    nc.vector.tensor_tensor(msqcol[:], mcol[:], sqcol[:], op=OP.mult)

    ps_d2 = psaux.tile([16, N], F32, tag="aux")
    for cg in range(NCG):
        for sblk in range(4):
            mlh = work.tile([128, 16], BF16, tag="mlh")
            nc.vector.tensor_scalar(
                mlh[:], csel(cg),
                msqcol[:, sblk * 16 + cg: sblk * 16 + cg + 1], None,
                op0=OP.mult)
            nc.tensor.matmul(
                ps_d2[:], lhsT=mlh[:],
                rhs=Call[:, cg * 2048 + sblk * 512: cg * 2048 + (sblk + 1) * 512],
                start=(cg == 0 and sblk == 0),
                stop=(cg == NCG - 1 and sblk == 3))
    sqm_row = rows.tile([16, N], F32, tag="sqm")
    nc.vector.tensor_tensor(sqm_row[:], sq_row[:], mask_row[:], op=OP.mult)
    d2a = rows.tile([16, N], F32, tag="d2a")
    nc.vector.tensor_tensor(d2a[:], ps_d2[:], sqm_row[:], op=OP.mult)
    d2b = rows.tile([16, N], F32, tag="d2b")
    nc.vector.tensor_tensor(d2b[:], d2a[:], mask_row[:], op=OP.subtract)
    sq2_row = rows.tile([16, N], F32, tag="sq2")
    nc.scalar.activation(sq2_row[:], d2b[:], ACT.Sqrt, bias=1.0)
    rsd2_row = rows.tile([16, N], F32, tag="rsd2")
    nc.vector.reciprocal(rsd2_row[:], sq2_row[:])
    mr2_row = rows.tile([16, N], F32, tag="mr2")
    nc.vector.tensor_tensor(mr2_row[:], rsd2_row[:], mask_row[:], op=OP.mult)
    q_row = rows.tile([16, N], F32, tag="qrow")
    nc.vector.tensor_tensor(q_row[:], mr2_row[:], sq_row[:], op=OP.mult)
    gate2_row = rows.tile([16, N], F32, tag="gate2")
    nc.vector.tensor_tensor(gate2_row[:], sig_row[:], q_row[:], op=OP.mult)
    if DEBUG:
        nc.sync.dma_start(dbg["qrow"].ap(), q_row[:])

    qcol = const.tile([128, 64], F32, tag="qcol")
    g2col = const.tile([128, 64], F32, tag="g2col")
    for sblk in range(4):
        pt = pspt.tile([128, 128], F32, tag="pt")
        nc.tensor.transpose(pt[:, 0:16],
                            q_row[:, sblk * 128:(sblk + 1) * 128], id16)
        nc.vector.tensor_copy(qcol[:, sblk * 16:(sblk + 1) * 16], pt[:, 0:16])
        pt2 = pspt.tile([128, 128], F32, tag="pt")
        nc.tensor.transpose(pt2[:, 0:16],
                            gate2_row[:, sblk * 128:(sblk + 1) * 128], id16)
        nc.vector.tensor_copy(g2col[:, sblk * 16:(sblk + 1) * 16],
                              pt2[:, 0:16])

    # ---- pooled conv (node-major) + fused final attention pool ------------
    hpall = xTb  # xTb fully consumed by layer 1
    bfr = Bc("bfr")
    ps_mg = pscol.tile([128, 64], F32, tag="cols")  # cols 0:16 mean, 16:32 g
    mT2b = rows.tile([128, 16], BF16, tag="mT2b")
    c2b = rows.tile([128, 16], BF16, tag="c2b")
    a4 = rows.tile([128, 64], BF16, tag="a4")
    for cg in range(NCG):
        pxp = psmm.tile([128, 512], F32, tag="mm")
        for nt in range(4):
            for ci in range(3):
                nc.tensor.matmul(
                    pxp[:, nt * 128:(nt + 1) * 128],
                    lhsT=xcatT[:, cg * 1536 + ci * 512 + nt * 128:
                               cg * 1536 + ci * 512 + (nt + 1) * 128],
                    rhs=W(f"Wf{ci}"), start=(ci == 0), stop=(ci == 2))
        xwps = scr.tile([128, 512], BF16, tag="xwps")
        for nt in range(4):
            nc.scalar.activation(
                xwps[:, nt * 128:(nt + 1) * 128],
                pxp[:, nt * 128:(nt + 1) * 128], ACT.Copy,
                scale=g2col[:, nt * 16 + cg: nt * 16 + cg + 1])
        pm = psmm.tile([128, 512], F32, tag="mm")
        for dt in range(4):
            for sblk in range(4):
                nc.tensor.matmul(
                    pm[:, dt * 128:(dt + 1) * 128],
                    lhsT=Call[:, cg * 2048 + sblk * 512 + dt * 128:
                              cg * 2048 + sblk * 512 + (dt + 1) * 128],
                    rhs=xwps[:, sblk * 128:(sblk + 1) * 128],
                    start=(sblk == 0), stop=(sblk == 3))
        hp = hpall[:, cg * 512:(cg + 1) * 512]
        y2 = scr.tile([128, 512], BF16, tag="y2")
        for dt in range(4):
            nc.vector.scalar_tensor_tensor(
                y2[:, dt * 128:(dt + 1) * 128],
                pm[:, dt * 128:(dt + 1) * 128],
                qcol[:, dt * 16 + cg: dt * 16 + cg + 1], bfr,
                op0=OP.mult, op1=OP.add)
            nc.vector.tensor_scalar(
                hp[:, dt * 128:(dt + 1) * 128],
                y2[:, dt * 128:(dt + 1) * 128], 0.0,
                mcol[:, dt * 16 + cg: dt * 16 + cg + 1],
                op0=OP.max, op1=OP.mult)
        for dt in range(4):
            nc.tensor.matmul(ps_mg[:, cg:cg + 1],
                             lhsT=hp[:, dt * 128:(dt + 1) * 128],
                             rhs=onescol, start=(dt == 0), stop=(dt == 3))
        # per-graph c2 = tanh(mean @ Wg_fin); alpha2 via transposed hp
        pt2 = pspt.tile([128, 128], F32, tag="pt")
        nc.vector.tensor_scalar(mT2b[:, cg:cg + 1], ps_mg[:, cg:cg + 1],
                                1.0 / K1, None, op0=OP.mult)
        nc.tensor.matmul(pt2[:, 4:5], lhsT=W("Wgf"), rhs=mT2b[:, cg:cg + 1],
                         start=True, stop=True)
        nc.scalar.activation(c2b[:, cg:cg + 1], pt2[:, 4:5], ACT.Tanh)
        hpT = scr.tile([128, 512], BF16, tag="hpT")
        nc.sync.dma_start_transpose(
            hpT[:, 0:256].rearrange("p (c f) -> p c f", c=2, f=128),
            hp[:, 0:256])
        nc.sync.dma_start_transpose(
            hpT[:, 256:512].rearrange("p (c f) -> p c f", c=2, f=128),
            hp[:, 256:512])
        for dt in range(4):
            nc.tensor.matmul(pt2[:, dt:dt + 1],
                             lhsT=hpT[:, dt * 128:(dt + 1) * 128],
                             rhs=c2b[:, cg:cg + 1], start=True, stop=True)
        nc.scalar.activation(a4[:, cg * 4:(cg + 1) * 4], pt2[:, 0:4],
                             ACT.Sigmoid)
        for dt in range(4):
            nc.tensor.matmul(ps_mg[:, 16 + cg: 16 + cg + 1],
                             lhsT=hp[:, dt * 128:(dt + 1) * 128],
                             rhs=a4[:, cg * 4 + dt: cg * 4 + dt + 1],
                             start=(dt == 0), stop=(dt == 3))
    if DEBUG:
        nc.sync.dma_start(dbg["hp"].ap(), hpall[:])

    # ---- head MLP ----------------------------------------------------------
    pcat = rows.tile([128, 16], BF16, tag="pcat")
    nc.vector.tensor_copy(pcat[:], ps_mg[:, 16:32])
    p1 = pspt.tile([128, 128], F32, tag="pt")
    nc.tensor.matmul(p1[:, 0:NPC], lhsT=W("Wl1a"), rhs=pcat[:, 0:NPC],
                     start=True, stop=False)
    nc.tensor.matmul(p1[:, 0:NPC], lhsT=W("Wl1b"), rhs=pcat[:, NPC:2 * NPC],
                     start=False, stop=True)
    o1 = rows.tile([128, NPC], BF16, tag="o1")
    nc.scalar.activation(o1[:], p1[:, 0:NPC], ACT.Relu, bias=Bc("bl1col")[:])
    p2 = pspt.tile([128, 128], F32, tag="pt")
    nc.tensor.matmul(p2[0:64, 0:NPC], lhsT=W("Wl2"), rhs=o1[:], start=True,
                     stop=True)
    o2 = rows.tile([64, NPC], BF16, tag="o2")
    nc.scalar.activation(o2[:], p2[0:64, 0:NPC], ACT.Relu,
                         bias=Bc("bl2col")[0:64, :])
    p3 = pspt.tile([128, 128], F32, tag="pt")
    nc.tensor.matmul(p3[0:2, 0:NPC], lhsT=W("Wl3")[0:64, :], rhs=o2[:],
                     start=True, stop=True)
    o3 = rows.tile([2, NPC], F32, tag="o3")
    nc.vector.tensor_scalar(o3[:], p3[0:2, 0:NPC], Bc("bl3col")[0:2, :],
                            None, op0=OP.add)
    nc.sync.dma_start(t_out.ap(), o3[:])
    ctx.close()


_NC_CACHE = {}


def _get_nc():
    key = (DEBUG,)
    if key not in _NC_CACHE:
        _NC_CACHE[key] = _build()
    return _NC_CACHE[key]


def kernel(**inputs):
    in_maps = _host_prep(inputs)
    nc = _get_nc()
    res = run_bass_kernel_spmd(nc, in_maps, core_ids=list(range(NCORES)))
    out = np.empty((B, 2), np.float32)
    for c in range(NCORES):
        out[c * NPC:(c + 1) * NPC] = res.results[c]["out"].T
    kernel._last = res
    kernel._nc = nc
    return out


# revision 79
# speedup vs baseline: 1.0046x; 1.0006x over previous
"""Trainium2 Bass kernel for nn_CAGpool (GNN message passing, CAG pooling).

Sharding: data-parallel over the 64 graph pairs -> 8 pairs (16 component
graphs of 512 nodes) per NeuronCore.  Message passing is dense matmul
against a per-graph 512x512 adjacency DMA'd from host as raw edge counts
(A+I); degree rows likewise host-derived from the integer edge lists.
Symmetric norm folded into C on-device with fused scalar_tensor_tensor.

Reductions over features use N=1 matmul columns; reductions over nodes use
stt-accum ops; the top-256 threshold comes from gpsimd kth_largest.
"""

import os
import numpy as np
import ml_dtypes

import concourse.bass as bass
import concourse.tile as tile
from concourse import bacc, mybir
from concourse.bass_utils import run_bass_kernel_spmd

F32 = mybir.dt.float32
BF16 = mybir.dt.bfloat16

NCORES = 8
B = 64
NPC = B // NCORES          # graph pairs per core (8)
NCG = 2 * NPC              # component graphs per core (16)
N = 512                    # nodes per component graph
K1 = 256
DEBUG = bool(int(os.environ.get("KERNEL_DEBUG", "0")))



def _layout(ent):
    offs, off = {}, 0
    for nm, w in ent:
        offs[nm] = (off, w)
        off += w
    return offs, off


WOFF, WF_TOT = _layout(
    [("W1", 128), ("W2", 128), ("W3", 128), ("Wgf", 128)]
    + [(f"Wg{i}", 384) for i in range(3)]
    + [(f"Wal{i}", 768) for i in range(6)]
    + [(f"Wf{i}", 128) for i in range(3)]
    + [("Wl1a", 128), ("Wl1b", 128), ("Wl2", 64), ("Wl3", 2),
       ("ones", 128), ("csel", 256), ("rsel", 2048)])
BOFF, BF_TOT = _layout(
    [("bfr", 128), ("balcol", 6), ("bl1col", 1), ("bl2col", 1),
     ("bl3col", 1), ("identf", 128), ("bcols", 3), ("deg", 512),
     ("degcol", 64)])


def _host_prep(inputs):
    """Per-core input maps. Index-structure prep only: dense adjacency counts
    and degree counts come straight from the integer edge lists."""
    x = np.asarray(inputs["x"], np.float32)

    s_loc, d_loc = {}, {}
    for comp, (sk, dk) in enumerate((("src_c1", "dst_c1"),
                                     ("src_c2", "dst_c2"))):
        base = (np.arange(B) * N)[:, None]
        s_loc[comp] = (np.asarray(inputs[sk]).reshape(B, -1) - base).astype(np.int64)
        d_loc[comp] = (np.asarray(inputs[dk]).reshape(B, -1) - base).astype(np.int64)

    in_maps = []
    loop = np.arange(N, dtype=np.int64)
    for c in range(NCORES):
        xT = np.empty((128, NCG * N), np.float32)
        deg = np.zeros((16, N), np.float32)
        idx_parts = []
        for comp in range(2):
            for gl in range(NPC):
                g = c * NPC + gl
                cg = comp * NPC + gl
                r0 = g * 2 * N + comp * N
                xT[:, cg * N:(cg + 1) * N] = x[r0:r0 + N].T
                s = np.concatenate([s_loc[comp][g], loop])
                d = np.concatenate([d_loc[comp][g], loop])
                idx = (s & 127) * (NCG * 2048) + cg * 2048 + (s >> 7) * 512 + d
                idx_parts.append(idx)
                deg[cg] = np.bincount(d_loc[comp][g], minlength=N) + 1.0
        cnt = np.bincount(np.concatenate(idx_parts),
                          minlength=128 * NCG * 2048)
        cdense = cnt.astype(ml_dtypes.bfloat16).reshape(128, NCG * 2048)

        wpack = np.zeros((128, WF_TOT), np.float32)

        def put(nm, arr):
            o, w = WOFF[nm]
            arr = np.asarray(arr, np.float32)
            wpack[: arr.shape[0], o:o + arr.shape[1]] = arr

        put("W1", inputs["W1"]); put("W2", inputs["W2"]); put("W3", inputs["W3"])
        put("Wgf", inputs["Wg_fin"])
        for i in range(3):
            put(f"Wg{i}", np.asarray(inputs["Wg_att"])[i * 128:(i + 1) * 128])
        for i in range(6):
            put(f"Wal{i}", np.asarray(inputs["Wal"])[i * 128:(i + 1) * 128])
        for i in range(3):
            put(f"Wf{i}", np.asarray(inputs["Wf"])[i * 128:(i + 1) * 128])
        put("Wl1a", np.asarray(inputs["Wl1"])[:128])
        put("Wl1b", np.asarray(inputs["Wl1"])[128:])
        put("Wl2", inputs["Wl2"])
        put("Wl3", inputs["Wl3"])
        put("ones", np.ones((128, 128), np.float32))
        csel = np.zeros((128, 256), np.float32)
        for cg in range(NCG):
            csel[:, cg * 16 + cg] = 1.0
        put("csel", csel)
        rsel = np.zeros((16, 2048), np.float32)
        for cg in range(16):
            rsel[cg, cg * 128:(cg + 1) * 128] = 1.0
        put("rsel", rsel)

        bpack = np.zeros((128, BF_TOT), np.float32)

        def putb(nm, arr):
            o, w = BOFF[nm]
            arr = np.asarray(arr, np.float32)
            bpack[: arr.shape[0], o:o + arr.shape[1]] = arr

        putb("bfr", np.broadcast_to(np.asarray(inputs["bf"])[None, :],
                                    (128, 128)))
        putb("balcol", np.asarray(inputs["bal"]).reshape(6, 128).T)
        putb("bl1col", np.asarray(inputs["bl1"])[:, None])
        putb("bl2col", np.asarray(inputs["bl2"])[:, None])
        putb("bl3col", np.asarray(inputs["bl3"])[:, None])
        putb("bcols", np.stack([np.asarray(inputs["b1"]),
                                np.asarray(inputs["b2"]),
                                np.asarray(inputs["b3"])], 1))
        putb("identf", np.eye(128, dtype=np.float32))
        putb("deg", deg)
        degcol = np.empty((128, 64), np.float32)
        for sblk in range(4):
            degcol[:, sblk * 16:(sblk + 1) * 16] = \
                deg[:, sblk * 128:(sblk + 1) * 128].T
        putb("degcol", degcol)

        in_maps.append({"xT": np.ascontiguousarray(xT.astype(ml_dtypes.bfloat16)),
                        "cdense": np.ascontiguousarray(cdense),
                        "wpack": wpack.astype(ml_dtypes.bfloat16),
                        "wsel": rsel.astype(ml_dtypes.bfloat16),
                        "bpack": bpack})
    return in_maps


def _build():
    nc = bacc.Bacc("TRN2", target_bir_lowering=False, debug=False,
                   num_devices=NCORES)
    tin = {
        "xT": nc.dram_tensor("xT", [128, NCG * N], BF16, kind="ExternalInput"),
        "cdense": nc.dram_tensor("cdense", [128, NCG * 2048], BF16,
                                 kind="ExternalInput"),
        "wpack": nc.dram_tensor("wpack", [128, WF_TOT], BF16,
                                kind="ExternalInput"),
        "bpack": nc.dram_tensor("bpack", [128, BF_TOT], F32,
                                kind="ExternalInput"),
        "wsel": nc.dram_tensor("wsel", [16, 2048], BF16,
                               kind="ExternalInput"),
    }
    t_out = nc.dram_tensor("out", [2, NPC], F32, kind="ExternalOutput")
    dbg = {}
    if DEBUG:
        for nm, shape, dt in (
                ("C", [128, NCG * 2048], BF16), ("xcatT", [128, NCG * 1536], BF16),
                ("gpT", [128, 48], F32),
                ("scols", [128, 64], F32), ("thr", [1, 32], F32),
                ("mask", [16, N], F32), ("qrow", [16, N], F32),
                ("hp", [128, NCG * 512], BF16), ("pvTb", [128, 48], BF16)):
            dbg[nm] = nc.dram_tensor("dbg_" + nm, shape, dt,
                                     kind="ExternalOutput")
    with tile.TileContext(nc) as tc:
        _emit(nc, tc, tin, t_out, dbg)
    nc.compile()
    return nc


def _emit(nc, tc, tin, t_out, dbg):
    import contextlib
    ctx = contextlib.ExitStack()
    OP = mybir.AluOpType
    ACT = mybir.ActivationFunctionType

    const = ctx.enter_context(tc.tile_pool(name="const", bufs=1))
    rows = ctx.enter_context(tc.tile_pool(name="rows", bufs=1))
    work = ctx.enter_context(tc.tile_pool(name="work", bufs=4))
    scr = ctx.enter_context(tc.tile_pool(name="scr", bufs=3))
    psmm = ctx.enter_context(tc.tile_pool(name="psmm", bufs=4, space="PSUM"))
    pspt = ctx.enter_context(tc.tile_pool(name="pspt", bufs=2, space="PSUM"))
    pscol = ctx.enter_context(tc.tile_pool(name="pscol", bufs=1, space="PSUM"))
    psaux = ctx.enter_context(tc.tile_pool(name="psaux", bufs=1,
                                           space="PSUM"))

    wb = const.tile([128, WF_TOT], BF16, tag="wb")
    bp = const.tile([128, BF_TOT], F32, tag="bp")
    xTb = const.tile([128, NCG * N], BF16, tag="xTb")   # reused as hp later
    Call = const.tile([128, NCG * 2048], BF16, tag="Call")
    xcatT = const.tile([128, NCG * 1536], BF16, tag="xcatT")

    def W(nm):
        o, w = WOFF[nm]
        return wb[:, o:o + w]

    def Bc(nm):
        o, w = BOFF[nm]
        return bp[:, o:o + w]

    # all input DMAs on the sync queue: same-queue issue order is program
    # order, so the fold-critical loads (bp: deg/degcol; wsel: broadcast
    # selectors; first C chunks) land before the big weight/feature loads
    wsel = const.tile([16, 2048], BF16, tag="wsel")
    nc.sync.dma_start(bp[:], tin["bpack"].ap())
    nc.sync.dma_start(wsel[:], tin["wsel"].ap())
    for ck in range(1):
        nc.sync.dma_start(Call[:, ck * 4096:(ck + 1) * 4096],
                          tin["cdense"].ap()[:, ck * 4096:(ck + 1) * 4096])
    nc.sync.dma_start(wb[:], tin["wpack"].ap())
    nc.sync.dma_start(xTb[:], tin["xT"].ap())
    # remaining C chunks arrive per-2cg inside the fold/layer-1 loop below

    identf = Bc("identf")
    id16 = identf[0:16, 0:16]
    id1 = identf[0:1, 0:1]
    onescol = W("ones")[:, 0:1]

    def csel(cg):
        o, _ = WOFF["csel"]
        return wb[:, o + cg * 16: o + (cg + 1) * 16]

    def rself(cg):
        return wsel[0:16, cg * 128:(cg + 1) * 128]

    # ---- degree norm rows/cols --------------------------------------------
    deg_rows = Bc("deg")[0:16, :]
    sq_row = rows.tile([16, N], F32, tag="sq")
    nc.scalar.activation(sq_row[:], deg_rows, ACT.Sqrt)
    rsd_row = rows.tile([16, N], F32, tag="rsd")
    nc.vector.reciprocal(rsd_row[:], sq_row[:])
    rsd_rowb = rows.tile([16, N], BF16, tag="rsdb")
    nc.vector.tensor_copy(rsd_rowb[:], rsd_row[:])

    rsdcol = const.tile([128, 64], F32, tag="rsdcol")
    sqcol = const.tile([128, 64], F32, tag="sqcol")
    nc.scalar.activation(sqcol[:], Bc("degcol"), ACT.Sqrt)
    nc.vector.reciprocal(rsdcol[:], sqcol[:])

    # ---- fold norm into C + layer 1, pipelined behind chunked C DMA -------
    meanT = const.tile([128, 48], F32, tag="meanT")
    meanT2 = const.tile([128, 48], F32, tag="meanT2")
    zeros256 = const.tile([128, 256], BF16, tag="zeros256")
    nc.vector.memset(zeros256[:], 0.0)

    def layer(l, cg):
        wl = W(("W1", "W2", "W3")[l])
        bcol = Bc("bcols")[:, l:l + 1]
        pxw = psmm.tile([128, 512], F32, tag="mm")
        for nt in range(4):
            if l == 0:
                lhsT = xTb[:, cg * N + nt * 128: cg * N + (nt + 1) * 128]
            else:
                lhsT = xcatT[:, cg * 1536 + (l - 1) * 512 + nt * 128:
                             cg * 1536 + (l - 1) * 512 + (nt + 1) * 128]
            nc.tensor.matmul(pxw[:, nt * 128:(nt + 1) * 128], lhsT=lhsT,
                             rhs=wl, start=True, stop=True)
        xws = scr.tile([128, 512], BF16, tag="xws")
        if l == 0:
            nc.scalar.activation(xws[:], pxw[:], ACT.Copy)
        else:
            nc.vector.tensor_copy(xws[:, 0:256], pxw[:, 0:256])
            nc.scalar.activation(xws[:, 256:512], pxw[:, 256:512], ACT.Copy)
        ph = psmm.tile([128, 512], F32, tag="mm")
        for sblk in range(4):
            nc.tensor.matmul(
                ph[:],
                lhsT=xws[:, sblk * 128:(sblk + 1) * 128],
                rhs=Call[:, cg * 2048 + sblk * 512:
                         cg * 2048 + (sblk + 1) * 512],
                start=(sblk == 0), stop=(sblk == 3))
        out_sl = xcatT[:, cg * 1536 + l * 512: cg * 1536 + (l + 1) * 512]
        if l == 0:
            nc.scalar.activation(
                out_sl, ph[:], ACT.Relu, bias=bcol,
                accum_out=meanT[:, l * 16 + cg: l * 16 + cg + 1])
            nc.vector.memset(meanT2[:, l * 16 + cg: l * 16 + cg + 1], 0.0)
        else:
            nc.scalar.activation(
                out_sl[:, 0:256], ph[:, 0:256], ACT.Relu, bias=bcol,
                accum_out=meanT[:, l * 16 + cg: l * 16 + cg + 1])
            nc.vector.scalar_tensor_tensor(
                out_sl[:, 256:512], ph[:, 256:512], bcol, zeros256[:],
                op0=OP.add, op1=OP.max,
                accum_out=meanT2[:, l * 16 + cg: l * 16 + cg + 1])

    def fold_l0(cg):
        if cg % 2 == 0 and cg >= 2:
            nc.sync.dma_start(Call[:, cg * 2048:(cg + 2) * 2048],
                              tin["cdense"].ap()[:, cg * 2048:(cg + 2) * 2048])
        pbps = psmm.tile([128, N], F32, tag="mm")
        nc.tensor.matmul(pbps[:], lhsT=rself(cg), rhs=rsd_rowb[:],
                         start=True, stop=True)
        for sblk in range(4):
            sl = Call[:, cg * 2048 + sblk * 512: cg * 2048 + (sblk + 1) * 512]
            nc.vector.scalar_tensor_tensor(
                sl, sl, rsdcol[:, sblk * 16 + cg: sblk * 16 + cg + 1],
                pbps[:], op0=OP.mult, op1=OP.mult)
        layer(0, cg)

    # layers 2+3 interleaved (software-pipelined); per-cg after layer 3:
    # c = tanh(mean @ Wg), alpha cols, sigmoid, node-major xcat copy via
    # DMA transpose, then gp via N=1 matmuls
    meanTb = rows.tile([128, 48], BF16, tag="meanTb")
    msum = rows.tile([128, 48], F32, tag="msum")
    cTb = rows.tile([128, 48], BF16, tag="cTb")
    asigb = rows.tile([128, 64], BF16, tag="asigb")
    pca = pscol.tile([128, 64], F32, tag="cols")
    gpTp = psaux.tile([128, 64], F32, tag="aux")

    def attn(cg):
        nc.vector.tensor_tensor(msum[:, cg::16], meanT[:, cg::16],
                                meanT2[:, cg::16], op=OP.add)
        nc.vector.tensor_scalar(meanTb[:, cg::16], msum[:, cg::16],
                                1.0 / N, None, op0=OP.mult)
        pc = pspt.tile([128, 128], F32, tag="pt")
        for fo in range(3):
            for fi in range(3):
                nc.tensor.matmul(
                    pc[:, fo:fo + 1],
                    lhsT=W(f"Wg{fi}")[:, fo * 128:(fo + 1) * 128],
                    rhs=meanTb[:, fi * 16 + cg: fi * 16 + cg + 1],
                    start=(fi == 0), stop=(fi == 2))
        nc.scalar.activation(cTb[:, cg::16], pc[:, 0:3], ACT.Tanh)
        for nt in range(4):
            for ch in range(3):
                nc.tensor.matmul(
                    pca[:, nt * 16 + cg: nt * 16 + cg + 1],
                    lhsT=xcatT[:, cg * 1536 + ch * 512 + nt * 128:
                               cg * 1536 + ch * 512 + (nt + 1) * 128],
                    rhs=cTb[:, ch * 16 + cg: ch * 16 + cg + 1],
                    start=(ch == 0), stop=(ch == 2))
        nc.scalar.activation(asigb[:, cg::16], pca[:, cg::16], ACT.Sigmoid)
        xcN = scr.tile([128, 1536], BF16, tag="xcN")
        nc.sync.dma_start_transpose(
            xcN[:].rearrange("p (c f) -> p c f", c=12, f=128),
            xcatT[:, cg * 1536:(cg + 1) * 1536])
        for ch in range(3):
            for nt in range(4):
                nc.tensor.matmul(
                    gpTp[:, ch * 16 + cg: ch * 16 + cg + 1],
                    lhsT=xcN[:, (ch * 4 + nt) * 128:(ch * 4 + nt + 1) * 128],
                    rhs=asigb[:, nt * 16 + cg: nt * 16 + cg + 1],
                    start=(nt == 0), stop=(nt == 3))

    for i in range(NCG + 4):
        if i < NCG:
            fold_l0(i)
        if 2 <= i < NCG + 2:
            layer(1, i - 2)
        if i >= 4:
            layer(2, i - 4)
            attn(i - 4)
    if DEBUG:
        nc.sync.dma_start(dbg["C"].ap(), Call[:])
    if DEBUG:
        nc.sync.dma_start(dbg["xcatT"].ap(), xcatT[:])
        nc.sync.dma_start(dbg["gpT"].ap(), gpTp[:, 0:48])
    gpT = gpTp

    # ---- pv = att_lin(concat(gp1, gp2)) -----------------------------------
    gpcatTb = rows.tile([128, 48], BF16, tag="gpcatTb")
    for j in range(6):
        comp, ch = j // 3, j % 3
        nc.vector.tensor_copy(
            gpcatTb[:, j * 8:(j + 1) * 8],
            gpT[:, ch * 16 + comp * 8: ch * 16 + comp * 8 + 8])
    pvTb = rows.tile([128, 48], BF16, tag="pvTb")
    for co in range(6):
        pp = pspt.tile([128, 128], F32, tag="pt")
        for ci in range(6):
            nc.tensor.matmul(pp[:, 0:8],
                             lhsT=W(f"Wal{ci}")[:, co * 128:(co + 1) * 128],
                             rhs=gpcatTb[:, ci * 8:(ci + 1) * 8],
                             start=(ci == 0), stop=(ci == 5))
        nc.vector.tensor_scalar(pvTb[:, co * 8:(co + 1) * 8], pp[:, 0:8],
                                Bc("balcol")[:, co:co + 1], None, op0=OP.add)
    if DEBUG:
        nc.sync.dma_start(dbg["pvTb"].ap(), pvTb[:])

    # ---- 1/||pv|| per graph ------------------------------------------------
    pnn = pspt.tile([128, 128], F32, tag="pt")
    for j in range(16):
        comp, gl = j // 8, j % 8
        for ci in range(3):
            col = pvTb[:, (comp * 3 + ci) * 8 + gl: (comp * 3 + ci) * 8 + gl + 1]
            nc.tensor.matmul(pnn[0:1, j:j + 1], lhsT=col, rhs=col,
                             start=(ci == 0), stop=(ci == 2))
    nnrow = rows.tile([1, 16], F32, tag="nnrow")
    nc.vector.tensor_copy(nnrow[:], pnn[0:1, 0:16])
    sqnrow = rows.tile([1, 16], F32, tag="sqnrow")
    nc.scalar.activation(sqnrow[:], nnrow[:], ACT.Sqrt)
    rsnrow = rows.tile([1, 16], F32, tag="rsnrow")
    nc.vector.reciprocal(rsnrow[:], sqnrow[:])
    ptn = pspt.tile([128, 128], F32, tag="pt")
    nc.tensor.transpose(ptn[0:16, 0:1], rsnrow[:], id1)
    rsncol = rows.tile([16, 1], F32, tag="rsncol")
    nc.vector.tensor_copy(rsncol[:], ptn[0:16, 0:1])

    # ---- scores (cg-major cols for kth_largest) ---------------------------
    pcs = pscol.tile([128, 64], F32, tag="cols")
    for cg in range(NCG):
        comp, gl = cg // NPC, cg % NPC
        for nt in range(4):
            for ci in range(3):
                nc.tensor.matmul(
                    pcs[:, cg * 4 + nt: cg * 4 + nt + 1],
                    lhsT=xcatT[:, cg * 1536 + ci * 512 + nt * 128:
                               cg * 1536 + ci * 512 + (nt + 1) * 128],
                    rhs=pvTb[:, (comp * 3 + ci) * 8 + gl:
                             (comp * 3 + ci) * 8 + gl + 1],
                    start=(ci == 0), stop=(ci == 2))
    scols = rows.tile([128, 64], F32, tag="scols")
    for h in range(4):
        nc.vector.tensor_copy(scols[:, h * 16:(h + 1) * 16],
                              pcs[:, h * 16:(h + 1) * 16])
    if DEBUG:
        nc.sync.dma_start(dbg["scols"].ap(), scols[:])

    thr = rows.tile([1, 32], F32, tag="thr")
    for g in range(16):
        nc.gpsimd.kth_largest(thr[0:1, 2 * g:2 * g + 2],
                              scols[:, g * 4:(g + 1) * 4],
                              n_per_lane=4, k=256, quantile=0.5005)
    if DEBUG:
        nc.sync.dma_start(dbg["thr"].ap(), thr[:])
    ptt = pspt.tile([128, 128], F32, tag="pt")
    nc.tensor.transpose(ptt[0:16, 0:1], thr[0:1, 0::2], id1)
    thrcol = rows.tile([16, 1], F32, tag="thrcol")
    nc.vector.tensor_copy(thrcol[:], ptt[0:16, 0:1])

    score_row = rows.tile([16, N], F32, tag="score")
    sig_row = rows.tile([16, N], F32, tag="sig")
    for nt in range(4):
        pt = pspt.tile([128, 128], F32, tag="pt")
        nc.tensor.transpose(pt[0:16, :], scols[:, nt::4], identf)
        nc.vector.tensor_copy(score_row[:, nt * 128:(nt + 1) * 128],
                              pt[0:16, :])
        nc.scalar.activation(sig_row[:, nt * 128:(nt + 1) * 128],
                             pt[0:16, :], ACT.Sigmoid, scale=rsncol[:])

    mask_row = rows.tile([16, N], F32, tag="mask")
    nc.vector.tensor_scalar(mask_row[:], score_row[:], thrcol[:], None,
                            op0=OP.is_gt)
    if DEBUG:
        nc.sync.dma_start(dbg["mask"].ap(), mask_row[:])

    # ---- pooled degree -----------------------------------------------------
    mcol = const.tile([128, 64], F32, tag="mcol")
    for sblk in range(4):
        pt = pspt.tile([128, 128], F32, tag="pt")
        nc.tensor.transpose(pt[:, 0:16],
                            mask_row[:, sblk * 128:(sblk + 1) * 128], id16)
        nc.vector.tensor_copy(mcol[:, sblk * 16:(sblk + 1) * 16], pt[:, 0:16])
    msqcol = const.tile([128, 64], F32, tag="msqcol")
    nc.vector.tensor_tensor(msqcol[:], mcol[:], sqcol[:], op=OP.mult)

    ps_d2 = psaux.tile([16, N], F32, tag="aux")
    for cg in range(NCG):
        for sblk in range(4):
            mlh = work.tile([128, 16], BF16, tag="mlh")
            nc.vector.tensor_scalar(
                mlh[:], csel(cg),
                msqcol[:, sblk * 16 + cg: sblk * 16 + cg + 1], None,
                op0=OP.mult)
            nc.tensor.matmul(
                ps_d2[:], lhsT=mlh[:],
                rhs=Call[:, cg * 2048 + sblk * 512: cg * 2048 + (sblk + 1) * 512],
                start=(cg == 0 and sblk == 0),
                stop=(cg == NCG - 1 and sblk == 3))
    sqm_row = rows.tile([16, N], F32, tag="sqm")
    nc.vector.tensor_tensor(sqm_row[:], sq_row[:], mask_row[:], op=OP.mult)
    d2a = rows.tile([16, N], F32, tag="d2a")
    nc.vector.tensor_tensor(d2a[:], ps_d2[:], sqm_row[:], op=OP.mult)
    d2b = rows.tile([16, N], F32, tag="d2b")
    nc.vector.tensor_tensor(d2b[:], d2a[:], mask_row[:], op=OP.subtract)
    sq2_row = rows.tile([16, N], F32, tag="sq2")
    nc.scalar.activation(sq2_row[:], d2b[:], ACT.Sqrt, bias=1.0)
    rsd2_row = rows.tile([16, N], F32, tag="rsd2")
    nc.vector.reciprocal(rsd2_row[:], sq2_row[:])
    mr2_row = rows.tile([16, N], F32, tag="mr2")
    nc.vector.tensor_tensor(mr2_row[:], rsd2_row[:], mask_row[:], op=OP.mult)
    q_row = rows.tile([16, N], F32, tag="qrow")
    nc.vector.tensor_tensor(q_row[:], mr2_row[:], sq_row[:], op=OP.mult)
    gate2_row = rows.tile([16, N], F32, tag="gate2")
    nc.vector.tensor_tensor(gate2_row[:], sig_row[:], q_row[:], op=OP.mult)
    if DEBUG:
        nc.sync.dma_start(dbg["qrow"].ap(), q_row[:])

    qcol = const.tile([128, 64], F32, tag="qcol")
    g2col = const.tile([128, 64], F32, tag="g2col")
    for sblk in range(4):
        pt = pspt.tile([128, 128], F32, tag="pt")
        nc.tensor.transpose(pt[:, 0:16],
                            q_row[:, sblk * 128:(sblk + 1) * 128], id16)
        nc.vector.tensor_copy(qcol[:, sblk * 16:(sblk + 1) * 16], pt[:, 0:16])
        pt2 = pspt.tile([128, 128], F32, tag="pt")
        nc.tensor.transpose(pt2[:, 0:16],
                            gate2_row[:, sblk * 128:(sblk + 1) * 128], id16)
        nc.vector.tensor_copy(g2col[:, sblk * 16:(sblk + 1) * 16],
                              pt2[:, 0:16])

    # ---- pooled conv (node-major) + fused final attention pool ------------
    hpall = xTb  # xTb fully consumed by layer 1
    bfr = Bc("bfr")
    ps_mg = pscol.tile([128, 64], F32, tag="cols")  # cols 0:16 mean, 16:32 g
    mT2b = rows.tile([128, 16], BF16, tag="mT2b")
    c2b = rows.tile([128, 16], BF16, tag="c2b")
    a4 = rows.tile([128, 64], BF16, tag="a4")
    for cg in range(NCG):
        pxp = psmm.tile([128, 512], F32, tag="mm")
        for nt in range(4):
            for ci in range(3):
                nc.tensor.matmul(
                    pxp[:, nt * 128:(nt + 1) * 128],
                    lhsT=xcatT[:, cg * 1536 + ci * 512 + nt * 128:
                               cg * 1536 + ci * 512 + (nt + 1) * 128],
                    rhs=W(f"Wf{ci}"), start=(ci == 0), stop=(ci == 2))
        xwps = scr.tile([128, 512], BF16, tag="xwps")
        for nt in range(4):
            nc.scalar.activation(
                xwps[:, nt * 128:(nt + 1) * 128],
                pxp[:, nt * 128:(nt + 1) * 128], ACT.Copy,
                scale=g2col[:, nt * 16 + cg: nt * 16 + cg + 1])
        pm = psmm.tile([128, 512], F32, tag="mm")
        for dt in range(4):
            for sblk in range(4):
                nc.tensor.matmul(
                    pm[:, dt * 128:(dt + 1) * 128],
                    lhsT=Call[:, cg * 2048 + sblk * 512 + dt * 128:
                              cg * 2048 + sblk * 512 + (dt + 1) * 128],
                    rhs=xwps[:, sblk * 128:(sblk + 1) * 128],
                    start=(sblk == 0), stop=(sblk == 3))
        hp = hpall[:, cg * 512:(cg + 1) * 512]
        y2 = scr.tile([128, 512], BF16, tag="y2")
        for dt in range(4):
            nc.vector.scalar_tensor_tensor(
                y2[:, dt * 128:(dt + 1) * 128],
                pm[:, dt * 128:(dt + 1) * 128],
                qcol[:, dt * 16 + cg: dt * 16 + cg + 1], bfr,
                op0=OP.mult, op1=OP.add)
            nc.vector.tensor_scalar(
                hp[:, dt * 128:(dt + 1) * 128],
                y2[:, dt * 128:(dt + 1) * 128], 0.0,
                mcol[:, dt * 16 + cg: dt * 16 + cg + 1],
                op0=OP.max, op1=OP.mult)
        for dt in range(4):
            nc.tensor.matmul(ps_mg[:, cg:cg + 1],
                             lhsT=hp[:, dt * 128:(dt + 1) * 128],
                             rhs=onescol, start=(dt == 0), stop=(dt == 3))
        # per-graph c2 = tanh(mean @ Wg_fin); alpha2 via transposed hp
        pt2 = pspt.tile([128, 128], F32, tag="pt")
        nc.vector.tensor_scalar(mT2b[:, cg:cg + 1], ps_mg[:, cg:cg + 1],
                                1.0 / K1, None, op0=OP.mult)
        nc.tensor.matmul(pt2[:, 4:5], lhsT=W("Wgf"), rhs=mT2b[:, cg:cg + 1],
                         start=True, stop=True)
        nc.scalar.activation(c2b[:, cg:cg + 1], pt2[:, 4:5], ACT.Tanh)
        hpT = scr.tile([128, 512], BF16, tag="hpT")
        nc.sync.dma_start_transpose(
            hpT[:, 0:256].rearrange("p (c f) -> p c f", c=2, f=128),
            hp[:, 0:256])
        nc.sync.dma_start_transpose(
            hpT[:, 256:512].rearrange("p (c f) -> p c f", c=2, f=128),
            hp[:, 256:512])
        for dt in range(4):
            nc.tensor.matmul(pt2[:, dt:dt + 1],
                             lhsT=hpT[:, dt * 128:(dt + 1) * 128],
                             rhs=c2b[:, cg:cg + 1], start=True, stop=True)
        nc.scalar.activation(a4[:, cg * 4:(cg + 1) * 4], pt2[:, 0:4],
                             ACT.Sigmoid)
        for dt in range(4):
            nc.tensor.matmul(ps_mg[:, 16 + cg: 16 + cg + 1],
                             lhsT=hp[:, dt * 128:(dt + 1) * 128],
                             rhs=a4[:, cg * 4 + dt: cg * 4 + dt + 1],
                             start=(dt == 0), stop=(dt == 3))
    if DEBUG:
        nc.sync.dma_start(dbg["hp"].ap(), hpall[:])

    # ---- head MLP ----------------------------------------------------------
    pcat = rows.tile([128, 16], BF16, tag="pcat")
    nc.vector.tensor_copy(pcat[:], ps_mg[:, 16:32])
    p1 = pspt.tile([128, 128], F32, tag="pt")
    nc.tensor.matmul(p1[:, 0:NPC], lhsT=W("Wl1a"), rhs=pcat[:, 0:NPC],
                     start=True, stop=False)
    nc.tensor.matmul(p1[:, 0:NPC], lhsT=W("Wl1b"), rhs=pcat[:, NPC:2 * NPC],
                     start=False, stop=True)
    o1 = rows.tile([128, NPC], BF16, tag="o1")
    nc.scalar.activation(o1[:], p1[:, 0:NPC], ACT.Relu, bias=Bc("bl1col")[:])
    p2 = pspt.tile([128, 128], F32, tag="pt")
    nc.tensor.matmul(p2[0:64, 0:NPC], lhsT=W("Wl2"), rhs=o1[:], start=True,
                     stop=True)
    o2 = rows.tile([64, NPC], BF16, tag="o2")
    nc.scalar.activation(o2[:], p2[0:64, 0:NPC], ACT.Relu,
                         bias=Bc("bl2col")[0:64, :])
    p3 = pspt.tile([128, 128], F32, tag="pt")
    nc.tensor.matmul(p3[0:2, 0:NPC], lhsT=W("Wl3")[0:64, :], rhs=o2[:],
                     start=True, stop=True)
    o3 = rows.tile([2, NPC], F32, tag="o3")
    nc.vector.tensor_scalar(o3[:], p3[0:2, 0:NPC], Bc("bl3col")[0:2, :],
                            None, op0=OP.add)
    nc.sync.dma_start(t_out.ap(), o3[:])
    ctx.close()


_NC_CACHE = {}


def _get_nc():
    key = (DEBUG,)
    if key not in _NC_CACHE:
        _NC_CACHE[key] = _build()
    return _NC_CACHE[key]


def kernel(**inputs):
    in_maps = _host_prep(inputs)
    nc = _get_nc()
    res = run_bass_kernel_spmd(nc, in_maps, core_ids=list(range(NCORES)))
    out = np.empty((B, 2), np.float32)
    for c in range(NCORES):
        out[c * NPC:(c + 1) * NPC] = res.results[c]["out"].T
    kernel._last = res
    kernel._nc = nc
    return out


# revision 81
# speedup vs baseline: 1.0072x; 1.0026x over previous
"""Trainium2 Bass kernel for nn_CAGpool (GNN message passing, CAG pooling).

Sharding: data-parallel over the 64 graph pairs -> 8 pairs (16 component
graphs of 512 nodes) per NeuronCore.  Message passing is dense matmul
against a per-graph 512x512 adjacency DMA'd from host as raw edge counts
(A+I); degree rows likewise host-derived from the integer edge lists.
Symmetric norm folded into C on-device with fused scalar_tensor_tensor.

Reductions over features use N=1 matmul columns; reductions over nodes use
stt-accum ops; the top-256 threshold comes from gpsimd kth_largest.
"""

import os
import numpy as np
import ml_dtypes

import concourse.bass as bass
import concourse.tile as tile
from concourse import bacc, mybir
from concourse.bass_utils import run_bass_kernel_spmd

F32 = mybir.dt.float32
BF16 = mybir.dt.bfloat16

NCORES = 8
B = 64
NPC = B // NCORES          # graph pairs per core (8)
NCG = 2 * NPC              # component graphs per core (16)
N = 512                    # nodes per component graph
K1 = 256
DEBUG = bool(int(os.environ.get("KERNEL_DEBUG", "0")))



def _layout(ent):
    offs, off = {}, 0
    for nm, w in ent:
        offs[nm] = (off, w)
        off += w
    return offs, off


WOFF, WF_TOT = _layout(
    [("W1", 128), ("W2", 128), ("W3", 128), ("Wgf", 128)]
    + [(f"Wg{i}", 384) for i in range(3)]
    + [(f"Wal{i}", 768) for i in range(6)]
    + [(f"Wf{i}", 128) for i in range(3)]
    + [("Wl1a", 128), ("Wl1b", 128), ("Wl2", 64), ("Wl3", 2),
       ("ones", 128), ("csel", 256), ("rsel", 2048)])
BOFF, BF_TOT = _layout(
    [("bfr", 128), ("balcol", 6), ("bl1col", 1), ("bl2col", 1),
     ("bl3col", 1), ("identf", 128), ("bcols", 3), ("deg", 512),
     ("degcol", 64)])


def _host_prep(inputs):
    """Per-core input maps. Index-structure prep only: dense adjacency counts
    and degree counts come straight from the integer edge lists."""
    x = np.asarray(inputs["x"], np.float32)

    s_loc, d_loc = {}, {}
    for comp, (sk, dk) in enumerate((("src_c1", "dst_c1"),
                                     ("src_c2", "dst_c2"))):
        base = (np.arange(B) * N)[:, None]
        s_loc[comp] = (np.asarray(inputs[sk]).reshape(B, -1) - base).astype(np.int64)
        d_loc[comp] = (np.asarray(inputs[dk]).reshape(B, -1) - base).astype(np.int64)

    in_maps = []
    loop = np.arange(N, dtype=np.int64)
    for c in range(NCORES):
        xT = np.empty((128, NCG * N), np.float32)
        deg = np.zeros((16, N), np.float32)
        idx_parts = []
        for comp in range(2):
            for gl in range(NPC):
                g = c * NPC + gl
                cg = comp * NPC + gl
                r0 = g * 2 * N + comp * N
                xT[:, cg * N:(cg + 1) * N] = x[r0:r0 + N].T
                s = np.concatenate([s_loc[comp][g], loop])
                d = np.concatenate([d_loc[comp][g], loop])
                idx = (s & 127) * (NCG * 2048) + cg * 2048 + (s >> 7) * 512 + d
                idx_parts.append(idx)
                deg[cg] = np.bincount(d_loc[comp][g], minlength=N) + 1.0
        cnt = np.bincount(np.concatenate(idx_parts),
                          minlength=128 * NCG * 2048)
        cdense = cnt.astype(ml_dtypes.bfloat16).reshape(128, NCG * 2048)

        wpack = np.zeros((128, WF_TOT), np.float32)

        def put(nm, arr):
            o, w = WOFF[nm]
            arr = np.asarray(arr, np.float32)
            wpack[: arr.shape[0], o:o + arr.shape[1]] = arr

        put("W1", inputs["W1"]); put("W2", inputs["W2"]); put("W3", inputs["W3"])
        put("Wgf", inputs["Wg_fin"])
        for i in range(3):
            put(f"Wg{i}", np.asarray(inputs["Wg_att"])[i * 128:(i + 1) * 128])
        for i in range(6):
            put(f"Wal{i}", np.asarray(inputs["Wal"])[i * 128:(i + 1) * 128])
        for i in range(3):
            put(f"Wf{i}", np.asarray(inputs["Wf"])[i * 128:(i + 1) * 128])
        put("Wl1a", np.asarray(inputs["Wl1"])[:128])
        put("Wl1b", np.asarray(inputs["Wl1"])[128:])
        put("Wl2", inputs["Wl2"])
        put("Wl3", inputs["Wl3"])
        put("ones", np.ones((128, 128), np.float32))
        csel = np.zeros((128, 256), np.float32)
        for cg in range(NCG):
            csel[:, cg * 16 + cg] = 1.0
        put("csel", csel)
        rsel = np.zeros((16, 2048), np.float32)
        for cg in range(16):
            rsel[cg, cg * 128:(cg + 1) * 128] = 1.0
        put("rsel", rsel)

        bpack = np.zeros((128, BF_TOT), np.float32)

        def putb(nm, arr):
            o, w = BOFF[nm]
            arr = np.asarray(arr, np.float32)
            bpack[: arr.shape[0], o:o + arr.shape[1]] = arr

        putb("bfr", np.broadcast_to(np.asarray(inputs["bf"])[None, :],
                                    (128, 128)))
        putb("balcol", np.asarray(inputs["bal"]).reshape(6, 128).T)
        putb("bl1col", np.asarray(inputs["bl1"])[:, None])
        putb("bl2col", np.asarray(inputs["bl2"])[:, None])
        putb("bl3col", np.asarray(inputs["bl3"])[:, None])
        putb("bcols", np.stack([np.asarray(inputs["b1"]),
                                np.asarray(inputs["b2"]),
                                np.asarray(inputs["b3"])], 1))
        putb("identf", np.eye(128, dtype=np.float32))
        putb("deg", deg)
        degcol = np.empty((128, 64), np.float32)
        for sblk in range(4):
            degcol[:, sblk * 16:(sblk + 1) * 16] = \
                deg[:, sblk * 128:(sblk + 1) * 128].T
        putb("degcol", degcol)

        in_maps.append({"xT": np.ascontiguousarray(xT.astype(ml_dtypes.bfloat16)),
                        "cdense": np.ascontiguousarray(cdense),
                        "wpack": wpack.astype(ml_dtypes.bfloat16),
                        "wsel": rsel.astype(ml_dtypes.bfloat16),
                        "bpack": bpack})
    return in_maps


def _build():
    nc = bacc.Bacc("TRN2", target_bir_lowering=False, debug=False,
                   num_devices=NCORES)
    tin = {
        "xT": nc.dram_tensor("xT", [128, NCG * N], BF16, kind="ExternalInput"),
        "cdense": nc.dram_tensor("cdense", [128, NCG * 2048], BF16,
                                 kind="ExternalInput"),
        "wpack": nc.dram_tensor("wpack", [128, WF_TOT], BF16,
                                kind="ExternalInput"),
        "bpack": nc.dram_tensor("bpack", [128, BF_TOT], F32,
                                kind="ExternalInput"),
        "wsel": nc.dram_tensor("wsel", [16, 2048], BF16,
                               kind="ExternalInput"),
    }
    t_out = nc.dram_tensor("out", [2, NPC], F32, kind="ExternalOutput")
    dbg = {}
    if DEBUG:
        for nm, shape, dt in (
                ("C", [128, NCG * 2048], BF16), ("xcatT", [128, NCG * 1536], BF16),
                ("gpT", [128, 48], F32),
                ("scols", [128, 64], F32), ("thr", [1, 32], F32),
                ("mask", [16, N], F32), ("qrow", [16, N], F32),
                ("hp", [128, NCG * 512], BF16), ("pvTb", [128, 48], BF16)):
            dbg[nm] = nc.dram_tensor("dbg_" + nm, shape, dt,
                                     kind="ExternalOutput")
    with tile.TileContext(nc) as tc:
        _emit(nc, tc, tin, t_out, dbg)
    nc.compile()
    return nc


def _emit(nc, tc, tin, t_out, dbg):
    import contextlib
    ctx = contextlib.ExitStack()
    OP = mybir.AluOpType
    ACT = mybir.ActivationFunctionType

    const = ctx.enter_context(tc.tile_pool(name="const", bufs=1))
    rows = ctx.enter_context(tc.tile_pool(name="rows", bufs=1))
    work = ctx.enter_context(tc.tile_pool(name="work", bufs=4))
    scr = ctx.enter_context(tc.tile_pool(name="scr", bufs=3))
    psmm = ctx.enter_context(tc.tile_pool(name="psmm", bufs=4, space="PSUM"))
    pspt = ctx.enter_context(tc.tile_pool(name="pspt", bufs=2, space="PSUM"))
    pscol = ctx.enter_context(tc.tile_pool(name="pscol", bufs=1, space="PSUM"))
    psaux = ctx.enter_context(tc.tile_pool(name="psaux", bufs=1,
                                           space="PSUM"))

    wb = const.tile([128, WF_TOT], BF16, tag="wb")
    bp = const.tile([128, BF_TOT], F32, tag="bp")
    xTb = const.tile([128, NCG * N], BF16, tag="xTb")   # reused as hp later
    Call = const.tile([128, NCG * 2048], BF16, tag="Call")
    xcatT = const.tile([128, NCG * 1536], BF16, tag="xcatT")

    def W(nm):
        o, w = WOFF[nm]
        return wb[:, o:o + w]

    def Bc(nm):
        o, w = BOFF[nm]
        return bp[:, o:o + w]

    # all input DMAs on the sync queue: same-queue issue order is program
    # order, so the fold-critical loads (bp: deg/degcol; wsel: broadcast
    # selectors; first C chunks) land before the big weight/feature loads
    wsel = const.tile([16, 2048], BF16, tag="wsel")
    nc.sync.dma_start(bp[:], tin["bpack"].ap())
    nc.sync.dma_start(wsel[:], tin["wsel"].ap())
    for ck in range(1):
        nc.sync.dma_start(Call[:, ck * 4096:(ck + 1) * 4096],
                          tin["cdense"].ap()[:, ck * 4096:(ck + 1) * 4096])
    nc.sync.dma_start(wb[:], tin["wpack"].ap())
    nc.sync.dma_start(xTb[:], tin["xT"].ap())
    # remaining C chunks arrive per-2cg inside the fold/layer-1 loop below

    identf = Bc("identf")
    id16 = identf[0:16, 0:16]
    id1 = identf[0:1, 0:1]
    onescol = W("ones")[:, 0:1]

    def csel(cg):
        o, _ = WOFF["csel"]
        return wb[:, o + cg * 16: o + (cg + 1) * 16]

    def rself(cg):
        return wsel[0:16, cg * 128:(cg + 1) * 128]

    # ---- degree norm rows/cols --------------------------------------------
    deg_rows = Bc("deg")[0:16, :]
    sq_row = rows.tile([16, N], F32, tag="sq")
    nc.scalar.activation(sq_row[:], deg_rows, ACT.Sqrt)
    rsd_row = rows.tile([16, N], F32, tag="rsd")
    nc.vector.reciprocal(rsd_row[:], sq_row[:])
    rsd_rowb = rows.tile([16, N], BF16, tag="rsdb")
    nc.vector.tensor_copy(rsd_rowb[:], rsd_row[:])

    rsdcol = const.tile([128, 64], F32, tag="rsdcol")
    sqcol = const.tile([128, 64], F32, tag="sqcol")
    nc.scalar.activation(sqcol[:], Bc("degcol"), ACT.Sqrt)
    nc.vector.reciprocal(rsdcol[:], sqcol[:])

    # ---- fold norm into C + layer 1, pipelined behind chunked C DMA -------
    meanT = const.tile([128, 48], F32, tag="meanT")
    meanT2 = const.tile([128, 48], F32, tag="meanT2")
    zeros256 = const.tile([128, 256], BF16, tag="zeros256")
    nc.vector.memset(zeros256[:], 0.0)

    def layer(l, cg):
        wl = W(("W1", "W2", "W3")[l])
        bcol = Bc("bcols")[:, l:l + 1]
        pxw = psmm.tile([128, 512], F32, tag="mm")
        for nt in range(4):
            if l == 0:
                lhsT = xTb[:, cg * N + nt * 128: cg * N + (nt + 1) * 128]
            else:
                lhsT = xcatT[:, cg * 1536 + (l - 1) * 512 + nt * 128:
                             cg * 1536 + (l - 1) * 512 + (nt + 1) * 128]
            nc.tensor.matmul(pxw[:, nt * 128:(nt + 1) * 128], lhsT=lhsT,
                             rhs=wl, start=True, stop=True)
        xws = scr.tile([128, 512], BF16, tag="xws")
        if l == 0:
            nc.scalar.activation(xws[:], pxw[:], ACT.Copy)
        else:
            nc.vector.tensor_copy(xws[:, 0:256], pxw[:, 0:256])
            nc.scalar.activation(xws[:, 256:512], pxw[:, 256:512], ACT.Copy)
        ph = psmm.tile([128, 512], F32, tag="mm")
        for sblk in range(4):
            nc.tensor.matmul(
                ph[:],
                lhsT=xws[:, sblk * 128:(sblk + 1) * 128],
                rhs=Call[:, cg * 2048 + sblk * 512:
                         cg * 2048 + (sblk + 1) * 512],
                start=(sblk == 0), stop=(sblk == 3))
        out_sl = xcatT[:, cg * 1536 + l * 512: cg * 1536 + (l + 1) * 512]
        if l == 0:
            nc.scalar.activation(
                out_sl, ph[:], ACT.Relu, bias=bcol,
                accum_out=meanT[:, l * 16 + cg: l * 16 + cg + 1])
            nc.vector.memset(meanT2[:, l * 16 + cg: l * 16 + cg + 1], 0.0)
        else:
            nc.scalar.activation(
                out_sl[:, 0:256], ph[:, 0:256], ACT.Relu, bias=bcol,
                accum_out=meanT[:, l * 16 + cg: l * 16 + cg + 1])
            nc.vector.scalar_tensor_tensor(
                out_sl[:, 256:512], ph[:, 256:512], bcol, zeros256[:],
                op0=OP.add, op1=OP.max,
                accum_out=meanT2[:, l * 16 + cg: l * 16 + cg + 1])

    def fold_l0(cg):
        if cg % 2 == 0 and cg >= 2:
            nc.sync.dma_start(Call[:, cg * 2048:(cg + 2) * 2048],
                              tin["cdense"].ap()[:, cg * 2048:(cg + 2) * 2048])
        pbps = psmm.tile([128, N], F32, tag="mm")
        nc.tensor.matmul(pbps[:], lhsT=rself(cg), rhs=rsd_rowb[:],
                         start=True, stop=True)
        for sblk in range(4):
            sl = Call[:, cg * 2048 + sblk * 512: cg * 2048 + (sblk + 1) * 512]
            nc.vector.scalar_tensor_tensor(
                sl, sl, rsdcol[:, sblk * 16 + cg: sblk * 16 + cg + 1],
                pbps[:], op0=OP.mult, op1=OP.mult)
        layer(0, cg)

    # layers 2+3 interleaved (software-pipelined); per-cg after layer 3:
    # c = tanh(mean @ Wg), alpha cols, sigmoid, node-major xcat copy via
    # DMA transpose, then gp via N=1 matmuls
    meanTb = rows.tile([128, 48], BF16, tag="meanTb")
    msum = rows.tile([128, 48], F32, tag="msum")
    cTb = rows.tile([128, 48], BF16, tag="cTb")
    asigb = rows.tile([128, 64], BF16, tag="asigb")
    pca = pscol.tile([128, 64], F32, tag="cols")
    gpTp = psaux.tile([128, 64], F32, tag="aux")

    def attn(cg):
        nc.vector.tensor_tensor(msum[:, cg::16], meanT[:, cg::16],
                                meanT2[:, cg::16], op=OP.add)
        nc.vector.tensor_scalar(meanTb[:, cg::16], msum[:, cg::16],
                                1.0 / N, None, op0=OP.mult)
        pc = pspt.tile([128, 128], F32, tag="pt")
        for fo in range(3):
            for fi in range(3):
                nc.tensor.matmul(
                    pc[:, fo:fo + 1],
                    lhsT=W(f"Wg{fi}")[:, fo * 128:(fo + 1) * 128],
                    rhs=meanTb[:, fi * 16 + cg: fi * 16 + cg + 1],
                    start=(fi == 0), stop=(fi == 2))
        nc.scalar.activation(cTb[:, cg::16], pc[:, 0:3], ACT.Tanh)
        for nt in range(4):
            for ch in range(3):
                nc.tensor.matmul(
                    pca[:, nt * 16 + cg: nt * 16 + cg + 1],
                    lhsT=xcatT[:, cg * 1536 + ch * 512 + nt * 128:
                               cg * 1536 + ch * 512 + (nt + 1) * 128],
                    rhs=cTb[:, ch * 16 + cg: ch * 16 + cg + 1],
                    start=(ch == 0), stop=(ch == 2))
        nc.scalar.activation(asigb[:, cg::16], pca[:, cg::16], ACT.Sigmoid)
        xcN = scr.tile([128, 1536], BF16, tag="xcN")
        nc.sync.dma_start_transpose(
            xcN[:].rearrange("p (c f) -> p c f", c=12, f=128),
            xcatT[:, cg * 1536:(cg + 1) * 1536])
        for ch in range(3):
            for nt in range(4):
                nc.tensor.matmul(
                    gpTp[:, ch * 16 + cg: ch * 16 + cg + 1],
                    lhsT=xcN[:, (ch * 4 + nt) * 128:(ch * 4 + nt + 1) * 128],
                    rhs=asigb[:, nt * 16 + cg: nt * 16 + cg + 1],
                    start=(nt == 0), stop=(nt == 3))

    for i in range(NCG + 4):
        if i < NCG:
            fold_l0(i)
        if 2 <= i < NCG + 2:
            layer(1, i - 2)
        if i >= 4:
            layer(2, i - 4)
            attn(i - 4)
    if DEBUG:
        nc.sync.dma_start(dbg["C"].ap(), Call[:])
    if DEBUG:
        nc.sync.dma_start(dbg["xcatT"].ap(), xcatT[:])
        nc.sync.dma_start(dbg["gpT"].ap(), gpTp[:, 0:48])
    gpT = gpTp

    # ---- pv = att_lin(concat(gp1, gp2)) -----------------------------------
    gpcatTb = rows.tile([128, 48], BF16, tag="gpcatTb")
    for j in range(6):
        comp, ch = j // 3, j % 3
        nc.vector.tensor_copy(
            gpcatTb[:, j * 8:(j + 1) * 8],
            gpT[:, ch * 16 + comp * 8: ch * 16 + comp * 8 + 8])
    pvTb = rows.tile([128, 48], BF16, tag="pvTb")
    for co in range(6):
        pp = pspt.tile([128, 128], F32, tag="pt")
        for ci in range(6):
            nc.tensor.matmul(pp[:, 0:8],
                             lhsT=W(f"Wal{ci}")[:, co * 128:(co + 1) * 128],
                             rhs=gpcatTb[:, ci * 8:(ci + 1) * 8],
                             start=(ci == 0), stop=(ci == 5))
        nc.vector.tensor_scalar(pvTb[:, co * 8:(co + 1) * 8], pp[:, 0:8],
                                Bc("balcol")[:, co:co + 1], None, op0=OP.add)
    if DEBUG:
        nc.sync.dma_start(dbg["pvTb"].ap(), pvTb[:])

    # ---- 1/||pv|| per graph ------------------------------------------------
    pnn = pspt.tile([128, 128], F32, tag="pt")
    for j in range(16):
        comp, gl = j // 8, j % 8
        for ci in range(3):
            col = pvTb[:, (comp * 3 + ci) * 8 + gl: (comp * 3 + ci) * 8 + gl + 1]
            nc.tensor.matmul(pnn[0:1, j:j + 1], lhsT=col, rhs=col,
                             start=(ci == 0), stop=(ci == 2))
    nnrow = rows.tile([1, 16], F32, tag="nnrow")
    nc.vector.tensor_copy(nnrow[:], pnn[0:1, 0:16])
    sqnrow = rows.tile([1, 16], F32, tag="sqnrow")
    nc.scalar.activation(sqnrow[:], nnrow[:], ACT.Sqrt)
    rsnrow = rows.tile([1, 16], F32, tag="rsnrow")
    nc.vector.reciprocal(rsnrow[:], sqnrow[:])
    ptn = pspt.tile([128, 128], F32, tag="pt")
    nc.tensor.transpose(ptn[0:16, 0:1], rsnrow[:], id1)
    rsncol = rows.tile([16, 1], F32, tag="rsncol")
    nc.vector.tensor_copy(rsncol[:], ptn[0:16, 0:1])

    # ---- scores (cg-major cols for kth_largest) ---------------------------
    pcs = pscol.tile([128, 64], F32, tag="cols")
    for cg in range(NCG):
        comp, gl = cg // NPC, cg % NPC
        for nt in range(4):
            for ci in range(3):
                nc.tensor.matmul(
                    pcs[:, cg * 4 + nt: cg * 4 + nt + 1],
                    lhsT=xcatT[:, cg * 1536 + ci * 512 + nt * 128:
                               cg * 1536 + ci * 512 + (nt + 1) * 128],
                    rhs=pvTb[:, (comp * 3 + ci) * 8 + gl:
                             (comp * 3 + ci) * 8 + gl + 1],
                    start=(ci == 0), stop=(ci == 2))
    scols = rows.tile([128, 64], F32, tag="scols")
    for h in range(4):
        nc.vector.tensor_copy(scols[:, h * 16:(h + 1) * 16],
                              pcs[:, h * 16:(h + 1) * 16])
    if DEBUG:
        nc.sync.dma_start(dbg["scols"].ap(), scols[:])

    thr = rows.tile([1, 32], F32, tag="thr")
    for g in range(16):
        nc.gpsimd.kth_largest(thr[0:1, 2 * g:2 * g + 2],
                              scols[:, g * 4:(g + 1) * 4],
                              n_per_lane=4, k=256, quantile=0.5005)
    if DEBUG:
        nc.sync.dma_start(dbg["thr"].ap(), thr[:])
    ptt = pspt.tile([128, 128], F32, tag="pt")
    nc.tensor.transpose(ptt[0:16, 0:1], thr[0:1, 0::2], id1)
    thrcol = rows.tile([16, 1], F32, tag="thrcol")
    nc.vector.tensor_copy(thrcol[:], ptt[0:16, 0:1])

    score_row = rows.tile([16, N], F32, tag="score")
    sig_row = rows.tile([16, N], F32, tag="sig")
    for nt in range(4):
        pt = pspt.tile([128, 128], F32, tag="pt")
        nc.tensor.transpose(pt[0:16, :], scols[:, nt::4], identf)
        nc.vector.tensor_copy(score_row[:, nt * 128:(nt + 1) * 128],
                              pt[0:16, :])

    mask_row = rows.tile([16, N], F32, tag="mask")
    nc.vector.tensor_scalar(mask_row[:], score_row[:], thrcol[:], None,
                            op0=OP.is_gt)
    if DEBUG:
        nc.sync.dma_start(dbg["mask"].ap(), mask_row[:])

    # ---- pooled degree -----------------------------------------------------
    mcol = const.tile([128, 64], F32, tag="mcol")
    for sblk in range(4):
        pt = pspt.tile([128, 128], F32, tag="pt")
        nc.tensor.transpose(pt[:, 0:16],
                            mask_row[:, sblk * 128:(sblk + 1) * 128], id16)
        nc.vector.tensor_copy(mcol[:, sblk * 16:(sblk + 1) * 16], pt[:, 0:16])
    msqcol = const.tile([128, 64], F32, tag="msqcol")
    nc.vector.tensor_tensor(msqcol[:], mcol[:], sqcol[:], op=OP.mult)

    ps_d2 = psaux.tile([16, N], F32, tag="aux")
    for cg in range(NCG):
        for sblk in range(4):
            mlh = work.tile([128, 16], BF16, tag="mlh")
            nc.vector.tensor_scalar(
                mlh[:], csel(cg),
                msqcol[:, sblk * 16 + cg: sblk * 16 + cg + 1], None,
                op0=OP.mult)
            nc.tensor.matmul(
                ps_d2[:], lhsT=mlh[:],
                rhs=Call[:, cg * 2048 + sblk * 512: cg * 2048 + (sblk + 1) * 512],
                start=(cg == 0 and sblk == 0),
                stop=(cg == NCG - 1 and sblk == 3))
    sqm_row = rows.tile([16, N], F32, tag="sqm")
    nc.vector.tensor_tensor(sqm_row[:], sq_row[:], mask_row[:], op=OP.mult)
    d2a = rows.tile([16, N], F32, tag="d2a")
    nc.vector.tensor_tensor(d2a[:], ps_d2[:], sqm_row[:], op=OP.mult)
    d2b = rows.tile([16, N], F32, tag="d2b")
    nc.vector.tensor_tensor(d2b[:], d2a[:], mask_row[:], op=OP.subtract)
    sq2_row = rows.tile([16, N], F32, tag="sq2")
    nc.scalar.activation(sq2_row[:], d2b[:], ACT.Sqrt, bias=1.0)
    rsd2_row = rows.tile([16, N], F32, tag="rsd2")
    nc.vector.reciprocal(rsd2_row[:], sq2_row[:])
    mr2_row = rows.tile([16, N], F32, tag="mr2")
    nc.vector.tensor_tensor(mr2_row[:], rsd2_row[:], mask_row[:], op=OP.mult)
    q_row = rows.tile([16, N], F32, tag="qrow")
    nc.vector.tensor_tensor(q_row[:], mr2_row[:], sq_row[:], op=OP.mult)
    # sigmoid deferred here so both Sqrts (sqn, sq2) share one act-table era
    nc.scalar.activation(sig_row[:], score_row[:], ACT.Sigmoid,
                         scale=rsncol[:])
    gate2_row = rows.tile([16, N], F32, tag="gate2")
    nc.vector.tensor_tensor(gate2_row[:], sig_row[:], q_row[:], op=OP.mult)
    if DEBUG:
        nc.sync.dma_start(dbg["qrow"].ap(), q_row[:])

    qcol = const.tile([128, 64], F32, tag="qcol")
    g2col = const.tile([128, 64], F32, tag="g2col")
    for sblk in range(4):
        pt = pspt.tile([128, 128], F32, tag="pt")
        nc.tensor.transpose(pt[:, 0:16],
                            q_row[:, sblk * 128:(sblk + 1) * 128], id16)
        nc.vector.tensor_copy(qcol[:, sblk * 16:(sblk + 1) * 16], pt[:, 0:16])
        pt2 = pspt.tile([128, 128], F32, tag="pt")
        nc.tensor.transpose(pt2[:, 0:16],
                            gate2_row[:, sblk * 128:(sblk + 1) * 128], id16)
        nc.vector.tensor_copy(g2col[:, sblk * 16:(sblk + 1) * 16],
                              pt2[:, 0:16])

    # ---- pooled conv (node-major) + fused final attention pool ------------
    hpall = xTb  # xTb fully consumed by layer 1
    bfr = Bc("bfr")
    ps_mg = pscol.tile([128, 64], F32, tag="cols")  # cols 0:16 mean, 16:32 g
    mT2b = rows.tile([128, 16], BF16, tag="mT2b")
    c2b = rows.tile([128, 16], BF16, tag="c2b")
    a4 = rows.tile([128, 64], BF16, tag="a4")
    for cg in range(NCG):
        pxp = psmm.tile([128, 512], F32, tag="mm")
        for nt in range(4):
            for ci in range(3):
                nc.tensor.matmul(
                    pxp[:, nt * 128:(nt + 1) * 128],
                    lhsT=xcatT[:, cg * 1536 + ci * 512 + nt * 128:
                               cg * 1536 + ci * 512 + (nt + 1) * 128],
                    rhs=W(f"Wf{ci}"), start=(ci == 0), stop=(ci == 2))
        xwps = scr.tile([128, 512], BF16, tag="xwps")
        for nt in range(4):
            nc.scalar.activation(
                xwps[:, nt * 128:(nt + 1) * 128],
                pxp[:, nt * 128:(nt + 1) * 128], ACT.Copy,
                scale=g2col[:, nt * 16 + cg: nt * 16 + cg + 1])
        pm = psmm.tile([128, 512], F32, tag="mm")
        for dt in range(4):
            for sblk in range(4):
                nc.tensor.matmul(
                    pm[:, dt * 128:(dt + 1) * 128],
                    lhsT=Call[:, cg * 2048 + sblk * 512 + dt * 128:
                              cg * 2048 + sblk * 512 + (dt + 1) * 128],
                    rhs=xwps[:, sblk * 128:(sblk + 1) * 128],
                    start=(sblk == 0), stop=(sblk == 3))
        hp = hpall[:, cg * 512:(cg + 1) * 512]
        y2 = scr.tile([128, 512], BF16, tag="y2")
        for dt in range(4):
            nc.vector.scalar_tensor_tensor(
                y2[:, dt * 128:(dt + 1) * 128],
                pm[:, dt * 128:(dt + 1) * 128],
                qcol[:, dt * 16 + cg: dt * 16 + cg + 1], bfr,
                op0=OP.mult, op1=OP.add)
            nc.vector.tensor_scalar(
                hp[:, dt * 128:(dt + 1) * 128],
                y2[:, dt * 128:(dt + 1) * 128], 0.0,
                mcol[:, dt * 16 + cg: dt * 16 + cg + 1],
                op0=OP.max, op1=OP.mult)
        for dt in range(4):
            nc.tensor.matmul(ps_mg[:, cg:cg + 1],
                             lhsT=hp[:, dt * 128:(dt + 1) * 128],
                             rhs=onescol, start=(dt == 0), stop=(dt == 3))
        # per-graph c2 = tanh(mean @ Wg_fin); alpha2 via transposed hp
        pt2 = pspt.tile([128, 128], F32, tag="pt")
        nc.vector.tensor_scalar(mT2b[:, cg:cg + 1], ps_mg[:, cg:cg + 1],
                                1.0 / K1, None, op0=OP.mult)
        nc.tensor.matmul(pt2[:, 4:5], lhsT=W("Wgf"), rhs=mT2b[:, cg:cg + 1],
                         start=True, stop=True)
        nc.scalar.activation(c2b[:, cg:cg + 1], pt2[:, 4:5], ACT.Tanh)
        hpT = scr.tile([128, 512], BF16, tag="hpT")
        nc.sync.dma_start_transpose(
            hpT[:, 0:256].rearrange("p (c f) -> p c f", c=2, f=128),
            hp[:, 0:256])
        nc.sync.dma_start_transpose(
            hpT[:, 256:512].rearrange("p (c f) -> p c f", c=2, f=128),
            hp[:, 256:512])
        for dt in range(4):
            nc.tensor.matmul(pt2[:, dt:dt + 1],
                             lhsT=hpT[:, dt * 128:(dt + 1) * 128],
                             rhs=c2b[:, cg:cg + 1], start=True, stop=True)
        nc.scalar.activation(a4[:, cg * 4:(cg + 1) * 4], pt2[:, 0:4],
                             ACT.Sigmoid)
        for dt in range(4):
            nc.tensor.matmul(ps_mg[:, 16 + cg: 16 + cg + 1],
                             lhsT=hp[:, dt * 128:(dt + 1) * 128],
                             rhs=a4[:, cg * 4 + dt: cg * 4 + dt + 1],
                             start=(dt == 0), stop=(dt == 3))
    if DEBUG:
        nc.sync.dma_start(dbg["hp"].ap(), hpall[:])

    # ---- head MLP ----------------------------------------------------------
    pcat = rows.tile([128, 16], BF16, tag="pcat")
    nc.vector.tensor_copy(pcat[:], ps_mg[:, 16:32])
    p1 = pspt.tile([128, 128], F32, tag="pt")
    nc.tensor.matmul(p1[:, 0:NPC], lhsT=W("Wl1a"), rhs=pcat[:, 0:NPC],
                     start=True, stop=False)
    nc.tensor.matmul(p1[:, 0:NPC], lhsT=W("Wl1b"), rhs=pcat[:, NPC:2 * NPC],
                     start=False, stop=True)
    o1 = rows.tile([128, NPC], BF16, tag="o1")
    nc.scalar.activation(o1[:], p1[:, 0:NPC], ACT.Relu, bias=Bc("bl1col")[:])
    p2 = pspt.tile([128, 128], F32, tag="pt")
    nc.tensor.matmul(p2[0:64, 0:NPC], lhsT=W("Wl2"), rhs=o1[:], start=True,
                     stop=True)
    o2 = rows.tile([64, NPC], BF16, tag="o2")
    nc.scalar.activation(o2[:], p2[0:64, 0:NPC], ACT.Relu,
                         bias=Bc("bl2col")[0:64, :])
    p3 = pspt.tile([128, 128], F32, tag="pt")
    nc.tensor.matmul(p3[0:2, 0:NPC], lhsT=W("Wl3")[0:64, :], rhs=o2[:],
                     start=True, stop=True)
    o3 = rows.tile([2, NPC], F32, tag="o3")
    nc.vector.tensor_scalar(o3[:], p3[0:2, 0:NPC], Bc("bl3col")[0:2, :],
                            None, op0=OP.add)
    nc.sync.dma_start(t_out.ap(), o3[:])
    ctx.close()


_NC_CACHE = {}


def _get_nc():
    key = (DEBUG,)
    if key not in _NC_CACHE:
        _NC_CACHE[key] = _build()
    return _NC_CACHE[key]


def kernel(**inputs):
    in_maps = _host_prep(inputs)
    nc = _get_nc()
    res = run_bass_kernel_spmd(nc, in_maps, core_ids=list(range(NCORES)))
    out = np.empty((B, 2), np.float32)
    for c in range(NCORES):
        out[c * NPC:(c + 1) * NPC] = res.results[c]["out"].T
    kernel._last = res
    kernel._nc = nc
    return out
